# revision 2
# baseline (speedup 1.0000x reference)
"""Trainium2 Bass kernel for the ButterflyMlp problem.

Computes log_softmax(L3(relu(L2(relu(L1(x)))))) where each Li is a masked
linear layer (butterfly sparsity: global column stripes + a diagonal band),
batch 65536, data-parallel over 8 NeuronCores (8192 rows/core).

Strategy (per core, feature-major throughout):
  - Masks are pre-applied to weights on host. Layer-1 exploits the butterfly
    structure: the stripe columns (mask true for every output row) form a
    dense [|S|, 784] GEMM shared by all outputs, and the per-output-block
    band adds one narrow [|R_j|<=128, 128] GEMM per 128-row output block.
    21 matmul passes per 512-batch chunk instead of 49 dense.
  - Output blocks are 128 wide (6x128+16) so 20/21 layer-1 matmuls load a
    full 128-column stationary (FWL-eligible at fp16).
  - PSUM tiles are 2 banks wide ([*, 1024]); each relu+bias eviction covers
    a 1024-column chunk pair, halving ACT/DVE per-op overhead.
  - log_softmax is batched 4 chunks at a time: the four [10, 512] layer-3
    GEMMs are col-tiled (tile_position=(0, 32*t)) into one PSUM bank at
    partition offsets 0/32/64/96 and run concurrently; b3 is accumulated
    into the same bank by a K=1 ones-matmul; one block-diagonal ones-matmul
    computes all four groups' sum(exp) at once. Exp/Ln then process
    [106, 512] (4 chunks) per op instead of [10, 512] per chunk, and the
    final subtract reads PSUM directly (no y3 eviction op at all).
  - All x data for a superchunk moves in ONE SWDGE transfer (stripe rows +
    band rows packed into a single host-side slab) to cut per-transfer
    GpSimd descriptor-generation; weights move in one SWDGE transfer.
    The first two superchunks are small (512 cols) so the first matmul
    fires ~6us in instead of ~20us.
"""
import sys
sys.path.insert(0, "/opt/trn_rl_repo")
import numpy as np

import concourse.bass as bass
import concourse.bacc as bacc
import concourse.mybir as mybir
import concourse.tile as tile
from concourse import bass_utils

import os
F32 = mybir.dt.float32
_MM_DT_NAME = os.environ.get("BUTTERFLY_MM_DT", "float16")
F32R = getattr(mybir.dt, _MM_DT_NAME)      # matmul operand dtype
_MM_NP = {"float32r": np.float32, "float16": np.float16,
          "bfloat16": None}[_MM_DT_NAME]
F16 = mybir.dt.float16
AF = mybir.ActivationFunctionType
ALU = mybir.AluOpType

# All activation functions this kernel uses live together in the
# natural_log_exp_and_others table set, but the greedy per-function set
# chooser picks exp_and_others for Exp and natural_log* for Ln, reloading
# ACT tables twice per chunk (~1.3us each). Restrict every other set's
# advertised contents so the chooser lands on the one set that covers
# everything and emits a single load. Set ids stay valid: the dict keys
# and order are unchanged.
_PIN_SET = "natural_log_exp_and_others"
_orig_gat = bacc.get_activation_tables


def _pinned_gat(arch):
    tabs = _orig_gat(arch)
    need = {AF.Relu, AF.Identity, AF.Exp, AF.Ln, AF.Copy}
    if _PIN_SET in tabs and need <= tabs[_PIN_SET]:
        for name in tabs:
            if name != _PIN_SET:
                tabs[name] = tabs[name] - need
    return tabs


bacc.get_activation_tables = _pinned_gat

N_CORES = 8
NB = 512          # batch columns per matmul (one PSUM bank of fp32)
OT = 128          # layer-1 output block width (6x128 + 1x16)
SCS = [512, 512, 1024, 2048, 2048, 2048]   # superchunk widths (sum = Bc)
GRP = 4           # NB-chunks per log_softmax group (4 col-tile slots)


def _decompose_mask1(mask1):
    """Split the butterfly mask into stripe columns S (true for every row)
    and per-output-block residual columns R_j (blocks of OT rows)."""
    D_out, D_in = mask1.shape
    S = np.where(mask1.all(axis=0))[0]
    n_blk = (D_out + OT - 1) // OT
    stripe_set = np.zeros(D_in, dtype=bool)
    stripe_set[S] = True
    R_list = []
    for j in range(n_blk):
        blk = mask1[j * OT:(j + 1) * OT]
        cols = np.where(blk.any(axis=0) & ~stripe_set)[0]
        assert len(cols) <= 128, f"band block {j} has {len(cols)} cols"
        R_list.append(cols)
    return S, R_list


def _build_program(meta):
    nS, R_lens = meta["nS"], meta["R_lens"]
    P_pad = meta["P_pad"]
    Bc = meta["Bc"]
    D1, H, C = meta["D1"], meta["H"], meta["C"]
    n_blk = len(R_lens)
    blk_w = [min(OT, D1 - j * OT) for j in range(n_blk)]
    n_sc = (nS + 127) // 128              # stripe K-chunks
    sc_w = -(-nS // n_sc)                 # stripe chunk width (padded)
    n_lane = 2 + n_blk                    # x lanes per superchunk slab
    n_ch = Bc // NB                       # NB chunks per core
    assert sum(SCS) == Bc
    # chunk -> (superchunk index, local column offset)
    loc = []
    start = 0
    for s, S_w in enumerate(SCS):
        for co in range(0, S_w, NB):
            loc.append((s, co))
        start += S_w
    assert len(loc) == n_ch

    # wall column layout: [ws | wb | w2]
    ws_off, wb_off = 0, n_sc * D1
    w2_off = wb_off + D1
    wall_cols = w2_off + n_blk * H
    EP = 32 * (GRP - 1) + C               # epilogue partition span (106)

    nc = bacc.Bacc("TRN2", target_bir_lowering=False, debug=False,
                   enable_asserts=False, num_devices=N_CORES)

    xall_d = nc.dram_tensor("xall", [P_pad, n_lane * Bc], F32R,
                            kind="ExternalInput").ap()
    wall_d = nc.dram_tensor("wall", [128, wall_cols], F32R,
                            kind="ExternalInput").ap()
    w3_d = nc.dram_tensor("w3", [H, C], F32R, kind="ExternalInput").ap()
    b1_d = nc.dram_tensor("b1", [OT, n_blk], F32, kind="ExternalInput").ap()
    b2_d = nc.dram_tensor("b2", [H, 1], F32, kind="ExternalInput").ap()
    b3r_d = nc.dram_tensor("b3r", [1, EP], F32R, kind="ExternalInput").ap()
    onec_d = nc.dram_tensor("onec", [1, NB], F32R, kind="ExternalInput").ap()
    obd_d = nc.dram_tensor("obd", [EP, EP], F32R, kind="ExternalInput").ap()
    out_d = nc.dram_tensor("out", [C, Bc], F16, kind="ExternalOutput").ap()

    with tile.TileContext(nc) as tc:
        with tc.tile_pool(name="wp", bufs=1) as wp, \
             tc.tile_pool(name="xp", bufs=2) as xp, \
             tc.tile_pool(name="hp", bufs=2) as hp, \
             tc.tile_pool(name="yp", bufs=3) as yp, \
             tc.tile_pool(name="ep", bufs=2) as ep, \
             tc.tile_pool(name="ps1", bufs=2, space="PSUM") as ps1, \
             tc.tile_pool(name="ps2", bufs=1, space="PSUM") as ps2, \
             tc.tile_pool(name="ps3", bufs=1, space="PSUM") as ps3, \
             tc.tile_pool(name="ps4", bufs=1, space="PSUM") as ps4:

            # ---- resident weights: one SWDGE transfer for the big slab,
            # sync ring for the small epilogue constants.
            wall_sb = wp.tile([128, wall_cols], F32R)
            nc.gpsimd.dma_start(wall_sb[:], wall_d[:])
            w3_sb = wp.tile([H, C], F32R)
            nc.sync.dma_start(w3_sb[:], w3_d[:])
            b1_sb = wp.tile([OT, n_blk], F32)
            nc.sync.dma_start(b1_sb[:], b1_d[:])
            b2_sb = wp.tile([H, 1], F32)
            nc.sync.dma_start(b2_sb[:], b2_d[:])
            b3r_sb = wp.tile([1, EP], F32R)
            nc.sync.dma_start(b3r_sb[:], b3r_d[:])
            onec_sb = wp.tile([1, NB], F32R)
            nc.sync.dma_start(onec_sb[:], onec_d[:])
            obd_sb = wp.tile([EP, EP], F32R)
            nc.sync.dma_start(obd_sb[:], obd_d[:])

            # ---- emit every superchunk's x load up front (slot semaphores
            # throttle); ONE packed DMA per superchunk.
            x_tiles = []
            start = 0
            for s, S_w in enumerate(SCS):
                xt = xp.tile([P_pad, n_lane * S_w], F32R, name="xt", tag="xt")
                nc.gpsimd.dma_start(
                    xt[:], xall_d[:, n_lane * start:n_lane * (start + S_w)])
                x_tiles.append(xt)
                start += S_w

            y2_tiles = []
            for p in range(n_ch // 2):
                t0, t1 = 2 * p, 2 * p + 1
                halves = [loc[t0], loc[t1]]

                # ---- layer 1: 7 output blocks, PSUM tile spans the pair
                y1_t = []
                for j in range(n_blk):
                    wj = blk_w[j]
                    pt = ps1.tile([wj, 2 * NB], F32, tag="l1", name="p1")
                    for c in range(n_sc):
                        kw = nS - c * sc_w if c == n_sc - 1 else sc_w
                        for h, (s, co) in enumerate(halves):
                            S_w = SCS[s]
                            nc.tensor.matmul(
                                pt[:, h * NB:(h + 1) * NB],
                                wall_sb[:kw, ws_off + c * D1 + j * OT:
                                        ws_off + c * D1 + j * OT + wj],
                                x_tiles[s][:kw, c * S_w + co:
                                           c * S_w + co + NB],
                                start=(c == 0), stop=False)
                    for h, (s, co) in enumerate(halves):
                        S_w = SCS[s]
                        nc.tensor.matmul(
                            pt[:, h * NB:(h + 1) * NB],
                            wall_sb[:R_lens[j], wb_off + j * OT:
                                    wb_off + j * OT + wj],
                            x_tiles[s][:R_lens[j], (2 + j) * S_w + co:
                                       (2 + j) * S_w + co + NB],
                            start=False, stop=True)
                    h1 = hp.tile([wj, 2 * NB], F32R, name=f"y1_{j}",
                                 tag=f"y1{j}")
                    if j % 2 == 0:
                        nc.vector.tensor_scalar(h1[:], pt[:],
                                                b1_sb[:wj, j:j + 1], 0.0,
                                                op0=ALU.add, op1=ALU.max)
                    else:
                        nc.scalar.activation(h1[:], pt[:], AF.Relu,
                                             bias=b1_sb[:wj, j:j + 1])
                    y1_t.append(h1)

                # ---- layer 2 ----
                p2 = ps2.tile([H, 2 * NB], F32, tag="l2", name="p2")
                for k in range(n_blk):
                    wk = blk_w[k]
                    for h in range(2):
                        nc.tensor.matmul(
                            p2[:, h * NB:(h + 1) * NB],
                            wall_sb[:wk, w2_off + k * H:w2_off + (k + 1) * H],
                            y1_t[k][:, h * NB:(h + 1) * NB],
                            start=(k == 0), stop=(k == n_blk - 1))
                y2 = yp.tile([H, 2 * NB], F32R, tag="y2")
                nc.scalar.activation(y2[:], p2[:], AF.Relu,
                                     bias=b2_sb[:, 0:1])
                y2_tiles.append(y2)

                # ---- every 2 pairs: batched layer 3 + log_softmax for 4
                # chunks, col-tiled into one PSUM bank at partition
                # offsets 0/32/64/96.
                if p % 2 == 1:
                    g = p // 2
                    pg = ps3.tile([EP, NB], F32, tag="l3", name="pg")
                    # b3 broadcast into all slots via a K=1 ones-matmul
                    # (start=True clears the bank; col MMs accumulate).
                    nc.tensor.matmul(pg[:], b3r_sb[0:1, :], onec_sb[0:1, :],
                                     start=True, stop=False)
                    srcs = [(y2_tiles[p - 1], 0), (y2_tiles[p - 1], 1),
                            (y2_tiles[p], 0), (y2_tiles[p], 1)]
                    for tl, (y2t, h) in enumerate(srcs):
                        nc.tensor.matmul(
                            pg[32 * tl:32 * tl + C, :], w3_sb[:],
                            y2t[:, h * NB:(h + 1) * NB],
                            start=False, stop=(tl == 3),
                            tile_position=(0, 32 * tl))
                    ex = ep.tile([EP, NB], F32R, tag="ex")
                    nc.scalar.activation(ex[:], pg[:], AF.Exp)
                    ps_l = ps4.tile([EP, NB], F32, tag="lse", name="ps_l")
                    nc.tensor.matmul(ps_l[:], obd_sb[:], ex[:], start=True,
                                     stop=True)
                    ls = ep.tile([EP, NB], F32, tag="ls")
                    nc.scalar.activation(ls[:], ps_l[:], AF.Ln)
                    o = ep.tile([EP, NB], F16, tag="o")
                    nc.vector.tensor_tensor(o[:], pg[:], ls[:],
                                            op=ALU.subtract)
                    for tl in range(GRP):
                        t = GRP * g + tl
                        nc.scalar.dma_start(
                            out_d[:, t * NB:(t + 1) * NB],
                            o[32 * tl:32 * tl + C, :])

    nc.compile()
    return nc


_CACHE = {}


def _prepare(x, W1, b1, W2, b2, W3, b3, mask1, mask2, mask3):
    B, D1 = x.shape
    H = W2.shape[0]
    C = W3.shape[0]
    assert B % N_CORES == 0
    Bc = B // N_CORES

    S, R_list = _decompose_mask1(np.asarray(mask1))
    R_lens = [len(r) for r in R_list]
    n_blk = len(R_list)
    blk_w = [min(OT, D1 - j * OT) for j in range(n_blk)]
    P_pad = max(R_lens + [1])
    nS = len(S)
    n_sc = (nS + 127) // 128
    sc_w = -(-nS // n_sc)
    n_lane = 2 + n_blk
    EP = 32 * (GRP - 1) + C

    Wm1 = (np.asarray(W1) * np.asarray(mask1)).astype(np.float32)
    Wm2 = (np.asarray(W2) * np.asarray(mask2)).astype(np.float32)
    Wm3 = (np.asarray(W3) * np.asarray(mask3)).astype(np.float32)

    if _MM_NP is None:
        import ml_dtypes
        cast = lambda a: np.asarray(a, dtype=ml_dtypes.bfloat16)
    elif _MM_NP is np.float16:
        cast = lambda a: np.asarray(a, dtype=np.float16)
    else:
        cast = lambda a: np.asarray(a, dtype=np.float32)

    # ---- weight slab [128, ws | wb | w2] ----
    ws_off, wb_off = 0, n_sc * D1
    w2_off = wb_off + D1
    wall_cols = w2_off + n_blk * H
    wall = np.zeros((128, wall_cols), np.float32)
    for c in range(n_sc):
        rows = S[c * sc_w:(c + 1) * sc_w]
        wall[:len(rows), ws_off + c * D1:ws_off + (c + 1) * D1] = Wm1[:, rows].T
    for j, R in enumerate(R_list):
        wall[:len(R), wb_off + j * OT:wb_off + j * OT + blk_w[j]] = \
            Wm1[j * OT:j * OT + blk_w[j], R].T
    for k in range(n_blk):
        wall[:blk_w[k], w2_off + k * H:w2_off + (k + 1) * H] = \
            Wm2[:, k * OT:k * OT + blk_w[k]].T
    w3 = np.ascontiguousarray(Wm3.T)                      # [H, C]
    b1p = np.zeros((OT, n_blk), np.float32)
    for j in range(n_blk):
        b1p[:blk_w[j], j] = np.asarray(b1, np.float32)[j * OT:j * OT + blk_w[j]]
    b2p = np.asarray(b2, np.float32).reshape(H, 1)
    b3r = np.zeros((1, EP), np.float32)
    obd = np.zeros((EP, EP), np.float32)
    for tl in range(GRP):
        b3r[0, 32 * tl:32 * tl + C] = np.asarray(b3, np.float32)
        for m in range(32 * tl, min(32 * tl + 32, EP)):
            obd[32 * tl:32 * tl + C, m] = 1.0
    onec = np.ones((1, NB), np.float32)

    # ---- x slab: per core [P_pad, n_lane * Bc]; per superchunk s a
    # contiguous [P_pad, n_lane, S_w] block (stripe chunks then bands).
    xT = np.asarray(x, np.float32).T                      # [D1, B] view
    xall = np.zeros((N_CORES, P_pad, n_lane * Bc), np.float16
                    if _MM_NP is np.float16 else np.float32)
    if _MM_NP is None:
        import ml_dtypes
        xall = xall.astype(ml_dtypes.bfloat16)

    def fill_lane(lane, rows):
        data = cast(xT[rows]).reshape(len(rows), N_CORES, Bc)
        start = 0
        for s, S_w in enumerate(SCS):
            sl = data[:, :, start:start + S_w]             # [r, NC, S_w]
            dst = xall[:, :len(rows),
                       n_lane * start + lane * S_w:
                       n_lane * start + (lane + 1) * S_w]
            dst[:] = sl.transpose(1, 0, 2)
            start += S_w

    for c in range(n_sc):
        fill_lane(c, S[c * sc_w:(c + 1) * sc_w])
    for j, R in enumerate(R_list):
        fill_lane(2 + j, R)

    meta = dict(nS=nS, R_lens=R_lens, P_pad=P_pad, Bc=Bc, D1=D1, H=H, C=C)
    key = (B, D1, H, C, nS, tuple(R_lens), _MM_DT_NAME)
    if key not in _CACHE:
        _CACHE[key] = _build_program(meta)
    nc = _CACHE[key]

    in_maps = []
    for cidx in range(N_CORES):
        in_maps.append({
            "xall": xall[cidx],
            "wall": cast(wall), "w3": cast(w3),
            "b1": b1p, "b2": b2p,
            "b3r": cast(b3r), "onec": cast(onec), "obd": cast(obd),
        })
    return nc, in_maps, meta


def _assemble(results, meta):
    outs = [np.ascontiguousarray(results[c]["out"].T).astype(np.float32)
            for c in range(N_CORES)]
    return np.concatenate(outs, axis=0)


def kernel(**inputs):
    nc, in_maps, meta = _prepare(**inputs)
    res = bass_utils.run_bass_kernel_spmd(nc, in_maps,
                                          core_ids=list(range(N_CORES)))
    return _assemble(res.results, meta)


def kernel_traced(tmpdir=None, **inputs):
    """Same as kernel() but with NTFF profiling; returns (output, results)."""
    nc, in_maps, meta = _prepare(**inputs)
    res = bass_utils.run_bass_kernel_spmd(nc, in_maps,
                                          core_ids=list(range(N_CORES)),
                                          trace=True, tmpdir=tmpdir)
    return _assemble(res.results, meta), res


# revision 3
# speedup vs baseline: 1.3686x; 1.3686x over previous
"""Trainium2 Bass kernel for the ButterflyMlp problem.

Computes log_softmax(L3(relu(L2(relu(L1(x)))))) where each Li is a masked
linear layer (butterfly sparsity: global column stripes + a diagonal band),
batch 65536, data-parallel over 8 NeuronCores (8192 rows/core).

Strategy (per core, feature-major throughout):
  - Masks are pre-applied to weights on host. Layer-1 exploits the butterfly
    structure: the stripe columns (mask true for every output row) form a
    dense [|S|, 784] GEMM shared by all outputs, and the per-output-block
    band adds one narrow [|R_j|<=128, 128] GEMM per 128-row output block.
  - Layer-1 runs in fp8-e4m3 (x and masked W1 quantized; W1 scaled by 16 to
    stay in fp8 normal range, W2 divided by 16 to undo it — exact linear
    rescale, no extra ops). The 204 stripe columns contract in ONE matmul
    per output block via DoubleRow perf mode (2 fp8 k-rows per PE cell),
    halving the stripe matmul count; host-measured end-to-end error of the
    quantization is ~9e-3 vs the 2e-2 budget. fp8 also halves the x DMA
    stream (8.1 MB/core), which otherwise outruns SWDGE and stalls the PE.
  - PSUM tiles are 2 banks wide ([*, 1024]); each relu+bias eviction covers
    a 1024-column chunk pair, halving ACT/DVE per-op overhead.
  - log_softmax is batched 4 chunks at a time: the four [10, 512] layer-3
    GEMMs are col-tiled (tile_position=(0, 32*t)) into one PSUM bank at
    partition offsets 0/32/64/96 and run concurrently; b3 is accumulated
    into the same bank by a K=1 ones-matmul; one block-diagonal ones-matmul
    computes all four groups' sum(exp) at once. Exp/Ln then process
    [106, 512] (4 chunks) per op instead of [10, 512] per chunk, and the
    final subtract reads PSUM directly (no y3 eviction op at all).
  - All x data for a superchunk moves in ONE SWDGE transfer; the first two
    superchunks are small (512 cols) so the first matmul fires early.
"""
import sys
sys.path.insert(0, "/opt/trn_rl_repo")
import numpy as np
import ml_dtypes

import concourse.bass as bass
import concourse.bacc as bacc
import concourse.mybir as mybir
import concourse.tile as tile
from concourse import bass_utils

F32 = mybir.dt.float32
F16 = mybir.dt.float16
F8 = mybir.dt.float8e4
NP8 = ml_dtypes.float8_e4m3
AF = mybir.ActivationFunctionType
ALU = mybir.AluOpType
DR = mybir.MatmulPerfMode.DoubleRow

# All activation functions this kernel uses live together in the
# natural_log_exp_and_others table set, but the greedy per-function set
# chooser picks exp_and_others for Exp and natural_log* for Ln, reloading
# ACT tables twice per chunk (~1.3us each). Restrict every other set's
# advertised contents so the chooser lands on the one set that covers
# everything and emits a single load. Set ids stay valid: the dict keys
# and order are unchanged.
_PIN_SET = "natural_log_exp_and_others"
_orig_gat = bacc.get_activation_tables


def _pinned_gat(arch):
    tabs = _orig_gat(arch)
    need = {AF.Relu, AF.Identity, AF.Exp, AF.Ln, AF.Copy}
    if _PIN_SET in tabs and need <= tabs[_PIN_SET]:
        for name in tabs:
            if name != _PIN_SET:
                tabs[name] = tabs[name] - need
    return tabs


bacc.get_activation_tables = _pinned_gat

N_CORES = 8
NB = 512          # batch columns per matmul (one PSUM bank of fp32)
OT = 128          # layer-1 output block width (6x128 + 1x16)
SCS = [512, 512, 1024, 2048, 2048, 2048]   # superchunk widths (sum = Bc)
GRP = 4           # NB-chunks per log_softmax group (4 col-tile slots)
WSCALE = 16.0     # fp8 weight pre-scale (undone in W2)


def _decompose_mask1(mask1):
    """Split the butterfly mask into stripe columns S (true for every row)
    and per-output-block residual columns R_j (blocks of OT rows)."""
    D_out, D_in = mask1.shape
    S = np.where(mask1.all(axis=0))[0]
    n_blk = (D_out + OT - 1) // OT
    stripe_set = np.zeros(D_in, dtype=bool)
    stripe_set[S] = True
    R_list = []
    for j in range(n_blk):
        blk = mask1[j * OT:(j + 1) * OT]
        cols = np.where(blk.any(axis=0) & ~stripe_set)[0]
        assert len(cols) <= 128, f"band block {j} has {len(cols)} cols"
        R_list.append(cols)
    return S, R_list


def _build_program(meta):
    nS, R_lens = meta["nS"], meta["R_lens"]
    P_pad = meta["P_pad"]
    Bc = meta["Bc"]
    D1, H, C = meta["D1"], meta["H"], meta["C"]
    n_blk = len(R_lens)
    blk_w = [min(OT, D1 - j * OT) for j in range(n_blk)]
    n_sc = (nS + 127) // 128              # stripe K-chunks
    sc_w = -(-nS // n_sc)                 # stripe chunk width (padded)
    use_dr = (n_sc == 2)                  # DoubleRow wants exactly 2 chunks
    n_lane = n_sc + n_blk                 # x lanes per superchunk slab
    n_ch = Bc // NB                       # NB chunks per core
    assert sum(SCS) == Bc
    loc = []                              # chunk -> (superchunk, local col)
    for s, S_w in enumerate(SCS):
        for co in range(0, S_w, NB):
            loc.append((s, co))
    assert len(loc) == n_ch
    EP = 32 * (GRP - 1) + C               # epilogue partition span (106)

    nc = bacc.Bacc("TRN2", target_bir_lowering=False, debug=False,
                   enable_asserts=False, num_devices=N_CORES)

    x_d = [nc.dram_tensor(f"x{s}", [P_pad, n_lane, S_w], F8,
                          kind="ExternalInput").ap()
           for s, S_w in enumerate(SCS)]
    ws8_d = nc.dram_tensor("ws8", [sc_w, n_sc, D1], F8,
                           kind="ExternalInput").ap()
    wb8_d = nc.dram_tensor("wb8", [P_pad, D1], F8,
                           kind="ExternalInput").ap()
    w2_d = nc.dram_tensor("w2", [OT, n_blk * H], F16,
                          kind="ExternalInput").ap()
    w3_d = nc.dram_tensor("w3", [H, C], F16, kind="ExternalInput").ap()
    b1_d = nc.dram_tensor("b1", [OT, n_blk], F32, kind="ExternalInput").ap()
    b2_d = nc.dram_tensor("b2", [H, 1], F32, kind="ExternalInput").ap()
    b3r_d = nc.dram_tensor("b3r", [1, EP], F16, kind="ExternalInput").ap()
    onec_d = nc.dram_tensor("onec", [1, NB], F16, kind="ExternalInput").ap()
    obd_d = nc.dram_tensor("obd", [EP, EP], F16, kind="ExternalInput").ap()
    out_d = nc.dram_tensor("out", [C, Bc], F16, kind="ExternalOutput").ap()

    with tile.TileContext(nc) as tc:
        with tc.tile_pool(name="wp", bufs=1) as wp, \
             tc.tile_pool(name="xp", bufs=3) as xp, \
             tc.tile_pool(name="hp", bufs=2) as hp, \
             tc.tile_pool(name="yp", bufs=3) as yp, \
             tc.tile_pool(name="ep", bufs=2) as ep, \
             tc.tile_pool(name="ps1", bufs=2, space="PSUM") as ps1, \
             tc.tile_pool(name="ps2", bufs=1, space="PSUM") as ps2, \
             tc.tile_pool(name="ps3", bufs=1, space="PSUM") as ps3, \
             tc.tile_pool(name="ps4", bufs=1, space="PSUM") as ps4:

            # ---- resident weights (SWDGE for the three slabs, sync ring
            # for the small epilogue constants)
            ws8_sb = wp.tile([sc_w, n_sc, D1], F8)
            nc.gpsimd.dma_start(ws8_sb[:], ws8_d[:])
            wb8_sb = wp.tile([P_pad, D1], F8)
            nc.gpsimd.dma_start(wb8_sb[:], wb8_d[:])
            w2_sb = wp.tile([OT, n_blk * H], F16)
            nc.gpsimd.dma_start(w2_sb[:], w2_d[:])
            w3_sb = wp.tile([H, C], F16)
            nc.sync.dma_start(w3_sb[:], w3_d[:])
            b1_sb = wp.tile([OT, n_blk], F32)
            nc.sync.dma_start(b1_sb[:], b1_d[:])
            b2_sb = wp.tile([H, 1], F32)
            nc.sync.dma_start(b2_sb[:], b2_d[:])
            b3r_sb = wp.tile([1, EP], F16)
            nc.sync.dma_start(b3r_sb[:], b3r_d[:])
            onec_sb = wp.tile([1, NB], F16)
            nc.sync.dma_start(onec_sb[:], onec_d[:])
            obd_sb = wp.tile([EP, EP], F16)
            nc.sync.dma_start(obd_sb[:], obd_d[:])

            # ---- emit every superchunk's x load up front (slot semaphores
            # throttle); ONE packed DMA per superchunk.
            x_tiles = []
            for s, S_w in enumerate(SCS):
                xt = xp.tile([P_pad, n_lane, S_w], F8, name="xt", tag="xt")
                nc.gpsimd.dma_start(xt[:], x_d[s][:])
                x_tiles.append(xt)

            y2_tiles = []
            for p in range(n_ch // 2):
                t0, t1 = 2 * p, 2 * p + 1
                halves = [loc[t0], loc[t1]]

                # ---- layer 1: 7 output blocks, PSUM tile spans the pair
                y1_t = []
                for j in range(n_blk):
                    wj = blk_w[j]
                    pt = ps1.tile([wj, 2 * NB], F32, tag="l1", name="p1")
                    for h, (s, co) in enumerate(halves):
                        xt = x_tiles[s]
                        if use_dr:
                            nc.tensor.matmul(
                                pt[:, h * NB:(h + 1) * NB],
                                ws8_sb[:, :, j * OT:j * OT + wj],
                                xt[:sc_w, 0:n_sc, co:co + NB],
                                start=True, stop=False, perf_mode=DR)
                        else:
                            for c in range(n_sc):
                                kw = nS - c * sc_w if c == n_sc - 1 else sc_w
                                nc.tensor.matmul(
                                    pt[:, h * NB:(h + 1) * NB],
                                    ws8_sb[:kw, c, j * OT:j * OT + wj],
                                    xt[:kw, c:c + 1, co:co + NB],
                                    start=(c == 0), stop=False)
                        nc.tensor.matmul(
                            pt[:, h * NB:(h + 1) * NB],
                            wb8_sb[:R_lens[j], j * OT:j * OT + wj],
                            xt[:R_lens[j], n_sc + j:n_sc + j + 1, co:co + NB],
                            start=False, stop=True)
                    h1 = hp.tile([wj, 2 * NB], F16, name=f"y1_{j}",
                                 tag=f"y1{j}")
                    if j % 2 == 0:
                        nc.vector.tensor_scalar(h1[:], pt[:],
                                                b1_sb[:wj, j:j + 1], 0.0,
                                                op0=ALU.add, op1=ALU.max)
                    else:
                        nc.scalar.activation(h1[:], pt[:], AF.Relu,
                                             bias=b1_sb[:wj, j:j + 1])
                    y1_t.append(h1)

                # ---- layer 2 ----
                p2 = ps2.tile([H, 2 * NB], F32, tag="l2", name="p2")
                for k in range(n_blk):
                    wk = blk_w[k]
                    for h in range(2):
                        nc.tensor.matmul(
                            p2[:, h * NB:(h + 1) * NB],
                            w2_sb[:wk, k * H:(k + 1) * H],
                            y1_t[k][:, h * NB:(h + 1) * NB],
                            start=(k == 0), stop=(k == n_blk - 1))
                y2 = yp.tile([H, 2 * NB], F16, tag="y2")
                nc.scalar.activation(y2[:], p2[:], AF.Relu,
                                     bias=b2_sb[:, 0:1])
                y2_tiles.append(y2)

                # ---- every 2 pairs: batched layer 3 + log_softmax for 4
                # chunks, col-tiled into one PSUM bank at partition
                # offsets 0/32/64/96.
                if p % 2 == 1:
                    g = p // 2
                    pg = ps3.tile([EP, NB], F32, tag="l3", name="pg")
                    # b3 broadcast into all slots via a K=1 ones-matmul
                    # (start=True clears the bank; col MMs accumulate).
                    nc.tensor.matmul(pg[:], b3r_sb[0:1, :], onec_sb[0:1, :],
                                     start=True, stop=False)
                    srcs = [(y2_tiles[p - 1], 0), (y2_tiles[p - 1], 1),
                            (y2_tiles[p], 0), (y2_tiles[p], 1)]
                    for tl, (y2t, h) in enumerate(srcs):
                        nc.tensor.matmul(
                            pg[32 * tl:32 * tl + C, :], w3_sb[:],
                            y2t[:, h * NB:(h + 1) * NB],
                            start=False, stop=(tl == 3),
                            tile_position=(0, 32 * tl))
                    ex = ep.tile([EP, NB], F16, tag="ex")
                    nc.scalar.activation(ex[:], pg[:], AF.Exp)
                    ps_l = ps4.tile([EP, NB], F32, tag="lse", name="ps_l")
                    nc.tensor.matmul(ps_l[:], obd_sb[:], ex[:], start=True,
                                     stop=True)
                    ls = ep.tile([EP, NB], F32, tag="ls")
                    nc.scalar.activation(ls[:], ps_l[:], AF.Ln)
                    o = ep.tile([EP, NB], F16, tag="o")
                    nc.vector.tensor_tensor(o[:], pg[:], ls[:],
                                            op=ALU.subtract)
                    for tl in range(GRP):
                        t = GRP * g + tl
                        nc.scalar.dma_start(
                            out_d[:, t * NB:(t + 1) * NB],
                            o[32 * tl:32 * tl + C, :])

    nc.compile()
    return nc


_CACHE = {}


def _prepare(x, W1, b1, W2, b2, W3, b3, mask1, mask2, mask3):
    B, D1 = x.shape
    H = W2.shape[0]
    C = W3.shape[0]
    assert B % N_CORES == 0
    Bc = B // N_CORES

    S, R_list = _decompose_mask1(np.asarray(mask1))
    R_lens = [len(r) for r in R_list]
    n_blk = len(R_list)
    blk_w = [min(OT, D1 - j * OT) for j in range(n_blk)]
    P_pad = max(R_lens + [1])
    nS = len(S)
    n_sc = (nS + 127) // 128
    sc_w = -(-nS // n_sc)
    n_lane = n_sc + n_blk
    EP = 32 * (GRP - 1) + C

    Wm1 = (np.asarray(W1) * np.asarray(mask1)).astype(np.float32)
    Wm2 = (np.asarray(W2) * np.asarray(mask2)).astype(np.float32)
    Wm3 = (np.asarray(W3) * np.asarray(mask3)).astype(np.float32)

    c16 = lambda a: np.asarray(a, dtype=np.float16)
    c8 = lambda a: np.asarray(a, dtype=NP8)

    # ---- weights ----
    ws8 = np.zeros((sc_w, n_sc, D1), np.float32)
    for c in range(n_sc):
        rows = S[c * sc_w:(c + 1) * sc_w]
        ws8[:len(rows), c, :] = Wm1[:, rows].T * WSCALE
    wb8 = np.zeros((P_pad, D1), np.float32)
    for j, R in enumerate(R_list):
        wb8[:len(R), j * OT:j * OT + blk_w[j]] = \
            Wm1[j * OT:j * OT + blk_w[j], R].T * WSCALE
    w2 = np.zeros((OT, n_blk * H), np.float32)
    for k in range(n_blk):
        w2[:blk_w[k], k * H:(k + 1) * H] = \
            Wm2[:, k * OT:k * OT + blk_w[k]].T / WSCALE
    w3 = np.ascontiguousarray(Wm3.T)                      # [H, C]
    b1p = np.zeros((OT, n_blk), np.float32)
    for j in range(n_blk):
        b1p[:blk_w[j], j] = WSCALE * \
            np.asarray(b1, np.float32)[j * OT:j * OT + blk_w[j]]
    b2p = np.asarray(b2, np.float32).reshape(H, 1)
    b3r = np.zeros((1, EP), np.float32)
    obd = np.zeros((EP, EP), np.float32)
    for tl in range(GRP):
        b3r[0, 32 * tl:32 * tl + C] = np.asarray(b3, np.float32)
        for m in range(32 * tl, min(32 * tl + 32, EP)):
            obd[32 * tl:32 * tl + C, m] = 1.0
    onec = np.ones((1, NB), np.float32)

    # ---- x slabs: one array per superchunk [NC, P_pad, n_lane, S_w] ----
    xT = np.asarray(x, np.float32).T                      # [D1, B] view
    xarrs = [np.zeros((N_CORES, P_pad, n_lane, S_w), NP8) for S_w in SCS]

    def fill_lane(lane, rows):
        data = c8(xT[rows]).reshape(len(rows), N_CORES, Bc)
        start = 0
        for s, S_w in enumerate(SCS):
            xarrs[s][:, :len(rows), lane, :] = \
                data[:, :, start:start + S_w].transpose(1, 0, 2)
            start += S_w

    for c in range(n_sc):
        fill_lane(c, S[c * sc_w:(c + 1) * sc_w])
    for j, R in enumerate(R_list):
        fill_lane(n_sc + j, R)

    meta = dict(nS=nS, R_lens=R_lens, P_pad=P_pad, Bc=Bc, D1=D1, H=H, C=C)
    key = (B, D1, H, C, nS, tuple(R_lens))
    if key not in _CACHE:
        _CACHE[key] = _build_program(meta)
    nc = _CACHE[key]

    in_maps = []
    for cidx in range(N_CORES):
        m = {f"x{s}": xarrs[s][cidx] for s in range(len(SCS))}
        m.update({
            "ws8": c8(ws8), "wb8": c8(wb8), "w2": c16(w2), "w3": c16(w3),
            "b1": b1p, "b2": b2p,
            "b3r": c16(b3r), "onec": c16(onec), "obd": c16(obd),
        })
        in_maps.append(m)
    return nc, in_maps, meta


def _assemble(results, meta):
    outs = [np.ascontiguousarray(results[c]["out"].T).astype(np.float32)
            for c in range(N_CORES)]
    return np.concatenate(outs, axis=0)


def kernel(**inputs):
    nc, in_maps, meta = _prepare(**inputs)
    res = bass_utils.run_bass_kernel_spmd(nc, in_maps,
                                          core_ids=list(range(N_CORES)))
    return _assemble(res.results, meta)


def kernel_traced(tmpdir=None, **inputs):
    """Same as kernel() but with NTFF profiling; returns (output, results)."""
    nc, in_maps, meta = _prepare(**inputs)
    res = bass_utils.run_bass_kernel_spmd(nc, in_maps,
                                          core_ids=list(range(N_CORES)),
                                          trace=True, tmpdir=tmpdir)
    return _assemble(res.results, meta), res


# revision 13
# speedup vs baseline: 1.4512x; 1.0603x over previous
"""Trainium2 Bass kernel for the ButterflyMlp problem.

Computes log_softmax(L3(relu(L2(relu(L1(x)))))) where each Li is a masked
linear layer (butterfly sparsity: global column stripes + a diagonal band),
batch 65536, data-parallel over 8 NeuronCores (8192 rows/core).

Strategy (per core, feature-major throughout):
  - Masks are pre-applied to weights on host. Layer-1 exploits the butterfly
    structure: the stripe columns (mask true for every output row) form a
    dense [|S|, 784] GEMM shared by all outputs, and the per-output-block
    band adds one narrow [|R_j|<=128, 128] GEMM per 128-row output block.
  - Layers 1 and 2 run in fp8-e4m3 (x, masked W1, y1, and W2 quantized;
    W1 scaled by 16 and W2 by 8 to stay in fp8 normal range, the combined
    x128 undone for free by the ACT scale on the y2 eviction — exact linear
    rescale, no extra ops). The 204 stripe columns contract in ONE matmul
    per output block via DoubleRow perf mode (2 fp8 k-rows per PE cell),
    halving the stripe matmul count, and layer-2's 128-wide k-chunks pair
    up the same way (4 matmuls instead of 7). Host-measured end-to-end
    error of the full quantization is ~1.1e-2 vs the 2e-2 budget. fp8 also
    halves the x DMA stream (8.1 MB/core), which otherwise outruns SWDGE
    and stalls the PE.
  - PSUM tiles are 2 banks wide ([*, 1024]); each relu+bias eviction covers
    a 1024-column chunk pair, halving ACT/DVE per-op overhead.
  - log_softmax is batched 4 chunks at a time: the four [10, 512] layer-3
    GEMMs are col-tiled (tile_position=(0, 32*t)) into one PSUM bank at
    partition offsets 0/32/64/96 and run concurrently; b3 is accumulated
    into the same bank by a K=1 ones-matmul; one block-diagonal ones-matmul
    computes all four groups' sum(exp) at once. Exp/Ln then process
    [106, 512] (4 chunks) per op instead of [10, 512] per chunk, and the
    final subtract reads PSUM directly (no y3 eviction op at all).
  - All x data for a superchunk moves in ONE SWDGE transfer; the first two
    superchunks are small (512 cols) so the first matmul fires early.
"""
import sys
sys.path.insert(0, "/opt/trn_rl_repo")
import numpy as np
import ml_dtypes

import concourse.bass as bass
import concourse.bacc as bacc
import concourse.mybir as mybir
import concourse.tile as tile
from concourse import bass_utils

F32 = mybir.dt.float32
F16 = mybir.dt.float16
F8 = mybir.dt.float8e4
NP8 = ml_dtypes.float8_e4m3
AF = mybir.ActivationFunctionType
ALU = mybir.AluOpType
DR = mybir.MatmulPerfMode.DoubleRow

# All activation functions this kernel uses live together in the
# natural_log_exp_and_others table set, but the greedy per-function set
# chooser picks exp_and_others for Exp and natural_log* for Ln, reloading
# ACT tables twice per chunk (~1.3us each). Restrict every other set's
# advertised contents so the chooser lands on the one set that covers
# everything and emits a single load. Set ids stay valid: the dict keys
# and order are unchanged.
_PIN_SET = "natural_log_exp_and_others"
_orig_gat = bacc.get_activation_tables


def _pinned_gat(arch):
    tabs = _orig_gat(arch)
    need = {AF.Relu, AF.Identity, AF.Exp, AF.Ln, AF.Copy}
    if _PIN_SET in tabs and need <= tabs[_PIN_SET]:
        for name in tabs:
            if name != _PIN_SET:
                tabs[name] = tabs[name] - need
    return tabs


bacc.get_activation_tables = _pinned_gat

N_CORES = 8
NB = 512          # batch columns per matmul (one PSUM bank of fp32)
OT = 128          # layer-1 output block width (6x128 + 1x16)
SCS = [512, 512, 1024, 2048, 2048, 2048]   # superchunk widths (sum = Bc)
GRP = 4           # NB-chunks per log_softmax group (4 col-tile slots)
WSCALE = 16.0     # fp8 W1 pre-scale
W2SCALE = 8.0     # fp8 W2 pre-scale (y2 eviction divides by 16*8)


def _decompose_mask1(mask1):
    """Split the butterfly mask into stripe columns S (true for every row)
    and per-output-block residual columns R_j (blocks of OT rows)."""
    D_out, D_in = mask1.shape
    S = np.where(mask1.all(axis=0))[0]
    n_blk = (D_out + OT - 1) // OT
    stripe_set = np.zeros(D_in, dtype=bool)
    stripe_set[S] = True
    R_list = []
    for j in range(n_blk):
        blk = mask1[j * OT:(j + 1) * OT]
        cols = np.where(blk.any(axis=0) & ~stripe_set)[0]
        assert len(cols) <= 128, f"band block {j} has {len(cols)} cols"
        R_list.append(cols)
    return S, R_list


def _build_program(meta):
    nS, R_lens = meta["nS"], meta["R_lens"]
    P_pad = meta["P_pad"]
    Bc = meta["Bc"]
    D1, H, C = meta["D1"], meta["H"], meta["C"]
    n_blk = len(R_lens)
    blk_w = [min(OT, D1 - j * OT) for j in range(n_blk)]
    n_sc = (nS + 127) // 128              # stripe K-chunks
    sc_w = -(-nS // n_sc)                 # stripe chunk width (padded)
    use_dr = (n_sc == 2)                  # DoubleRow wants exactly 2 chunks
    n_lane = n_sc + n_blk                 # x lanes per superchunk slab
    n_ch = Bc // NB                       # NB chunks per core
    assert sum(SCS) == Bc
    loc = []                              # chunk -> (superchunk, local col)
    for s, S_w in enumerate(SCS):
        for co in range(0, S_w, NB):
            loc.append((s, co))
    assert len(loc) == n_ch
    EP = 32 * (GRP - 1) + C               # epilogue partition span (106)
    n_pr = n_blk // 2                     # layer-2 DoubleRow k-chunk pairs
    lw = blk_w[-1] if n_blk % 2 else 0    # leftover block width
    assert all(blk_w[2 * q] == OT and blk_w[2 * q + 1] == OT
               for q in range(n_pr)), "layer-2 DR pairs need full blocks"

    nc = bacc.Bacc("TRN2", target_bir_lowering=False, debug=False,
                   enable_asserts=False, num_devices=N_CORES)

    x_d = [nc.dram_tensor(f"x{s}", [P_pad, n_lane, S_w], F8,
                          kind="ExternalInput").ap()
           for s, S_w in enumerate(SCS)]
    ws8_d = nc.dram_tensor("ws8", [sc_w, n_sc, D1], F8,
                           kind="ExternalInput").ap()
    wb8_d = nc.dram_tensor("wb8", [P_pad, D1], F8,
                           kind="ExternalInput").ap()
    w2_d = nc.dram_tensor("w2", [OT, 2, n_pr * H], F8,
                          kind="ExternalInput").ap()
    w26_d = (nc.dram_tensor("w26", [lw, H], F8, kind="ExternalInput").ap()
             if lw else None)
    w3_d = nc.dram_tensor("w3", [H, C], F16, kind="ExternalInput").ap()
    b1_d = nc.dram_tensor("b1", [OT, n_blk], F32, kind="ExternalInput").ap()
    b2_d = nc.dram_tensor("b2", [H, 1], F32, kind="ExternalInput").ap()
    b3r_d = nc.dram_tensor("b3r", [1, EP], F16, kind="ExternalInput").ap()
    onec_d = nc.dram_tensor("onec", [1, NB], F16, kind="ExternalInput").ap()
    obd_d = nc.dram_tensor("obd", [EP, EP], F16, kind="ExternalInput").ap()
    out_d = nc.dram_tensor("out", [C, Bc], F16, kind="ExternalOutput").ap()

    with tile.TileContext(nc) as tc:
        with tc.tile_pool(name="wp", bufs=1) as wp, \
             tc.tile_pool(name="xp", bufs=3) as xp, \
             tc.tile_pool(name="hp", bufs=2) as hp, \
             tc.tile_pool(name="yp", bufs=3) as yp, \
             tc.tile_pool(name="ep", bufs=2) as ep, \
             tc.tile_pool(name="ps1", bufs=2, space="PSUM") as ps1, \
             tc.tile_pool(name="ps2", bufs=1, space="PSUM") as ps2, \
             tc.tile_pool(name="ps3", bufs=1, space="PSUM") as ps3, \
             tc.tile_pool(name="ps4", bufs=1, space="PSUM") as ps4:

            # ---- emit every superchunk's x load up front (slot semaphores
            # throttle); ONE packed SWDGE DMA per superchunk. SWDGE carries
            # ONLY x: weights ride the two HWDGE rings in parallel so the
            # x stream starts immediately and never queues behind them.
            x_tiles = []
            for s, S_w in enumerate(SCS):
                xt = xp.tile([P_pad, n_lane, S_w], F8, name="xt", tag="xt")
                nc.gpsimd.dma_start(xt[:], x_d[s][:])
                x_tiles.append(xt)

            # ---- resident weights on the sync/scalar HWDGE rings
            ws8_sb = wp.tile([sc_w, n_sc, D1], F8)
            nc.sync.dma_start(ws8_sb[:], ws8_d[:])
            wb8_sb = wp.tile([P_pad, D1], F8)
            nc.scalar.dma_start(wb8_sb[:], wb8_d[:])
            w2_sb = wp.tile([OT, 2, n_pr * H], F8)
            nc.scalar.dma_start(w2_sb[:], w2_d[:])
            if lw:
                w26_sb = wp.tile([lw, H], F8)
                nc.scalar.dma_start(w26_sb[:], w26_d[:])
            w3_sb = wp.tile([H, C], F16)
            nc.sync.dma_start(w3_sb[:], w3_d[:])
            b1_sb = wp.tile([OT, n_blk], F32)
            nc.sync.dma_start(b1_sb[:], b1_d[:])
            b2_sb = wp.tile([H, 1], F32)
            nc.sync.dma_start(b2_sb[:], b2_d[:])
            b3r_sb = wp.tile([1, EP], F16)
            nc.sync.dma_start(b3r_sb[:], b3r_d[:])
            onec_sb = wp.tile([1, NB], F16)
            nc.sync.dma_start(onec_sb[:], onec_d[:])
            obd_sb = wp.tile([EP, EP], F16)
            nc.sync.dma_start(obd_sb[:], obd_d[:])

            y2_tiles = []
            for p in range(n_ch // 2):
                t0, t1 = 2 * p, 2 * p + 1
                halves = [loc[t0], loc[t1]]

                # ---- layer 1: 7 output blocks, PSUM tile spans the pair;
                # evictions write fp8 into DoubleRow-paired k-chunk tiles
                y1p = [hp.tile([OT, 2, 2 * NB], F8, name=f"y1p_{q}",
                               tag=f"y1p{q}") for q in range(n_pr)]
                y1l = (hp.tile([lw, 2 * NB], F8, name="y1l", tag="y1l")
                       if lw else None)
                for j in range(n_blk):
                    wj = blk_w[j]
                    pt = ps1.tile([wj, 2 * NB], F32, tag="l1", name="p1")
                    for h, (s, co) in enumerate(halves):
                        xt = x_tiles[s]
                        if use_dr:
                            nc.tensor.matmul(
                                pt[:, h * NB:(h + 1) * NB],
                                ws8_sb[:, :, j * OT:j * OT + wj],
                                xt[:sc_w, 0:n_sc, co:co + NB],
                                start=True, stop=False, perf_mode=DR)
                        else:
                            for c in range(n_sc):
                                kw = nS - c * sc_w if c == n_sc - 1 else sc_w
                                nc.tensor.matmul(
                                    pt[:, h * NB:(h + 1) * NB],
                                    ws8_sb[:kw, c, j * OT:j * OT + wj],
                                    xt[:kw, c:c + 1, co:co + NB],
                                    start=(c == 0), stop=False)
                        nc.tensor.matmul(
                            pt[:, h * NB:(h + 1) * NB],
                            wb8_sb[:R_lens[j], j * OT:j * OT + wj],
                            xt[:R_lens[j], n_sc + j:n_sc + j + 1, co:co + NB],
                            start=False, stop=True)
                    dst = (y1p[j // 2][:wj, j % 2, :] if j < 2 * n_pr
                           else y1l[:])
                    if j % 2 == 0:
                        nc.vector.tensor_scalar(dst, pt[:],
                                                b1_sb[:wj, j:j + 1], 0.0,
                                                op0=ALU.add, op1=ALU.max)
                    else:
                        nc.scalar.activation(dst, pt[:], AF.Relu,
                                             bias=b1_sb[:wj, j:j + 1])

                # ---- layer 2 (fp8 DoubleRow over k-chunk pairs) ----
                p2 = ps2.tile([H, 2 * NB], F32, tag="l2", name="p2")
                for q in range(n_pr):
                    for h in range(2):
                        nc.tensor.matmul(
                            p2[:, h * NB:(h + 1) * NB],
                            w2_sb[:, :, q * H:(q + 1) * H],
                            y1p[q][:, :, h * NB:(h + 1) * NB],
                            start=(q == 0), stop=(q == n_pr - 1 and not lw),
                            perf_mode=DR)
                if lw:
                    for h in range(2):
                        nc.tensor.matmul(
                            p2[:, h * NB:(h + 1) * NB], w26_sb[:],
                            y1l[:, h * NB:(h + 1) * NB],
                            start=False, stop=True)
                # undo the fp8 weight pre-scales (x16 from W1, x8 from W2)
                y2 = yp.tile([H, 2 * NB], F16, tag="y2")
                nc.scalar.activation(y2[:], p2[:], AF.Relu,
                                     bias=b2_sb[:, 0:1],
                                     scale=1.0 / (WSCALE * W2SCALE))
                y2_tiles.append(y2)

                # ---- every 2 pairs: batched layer 3 + log_softmax for 4
                # chunks, col-tiled into one PSUM bank at partition
                # offsets 0/32/64/96.
                if p % 2 == 1:
                    g = p // 2
                    pg = ps3.tile([EP, NB], F32, tag="l3", name="pg")
                    # b3 broadcast into all slots via a K=1 ones-matmul
                    # (start=True clears the bank; col MMs accumulate).
                    nc.tensor.matmul(pg[:], b3r_sb[0:1, :], onec_sb[0:1, :],
                                     start=True, stop=False)
                    srcs = [(y2_tiles[p - 1], 0), (y2_tiles[p - 1], 1),
                            (y2_tiles[p], 0), (y2_tiles[p], 1)]
                    for tl, (y2t, h) in enumerate(srcs):
                        nc.tensor.matmul(
                            pg[32 * tl:32 * tl + C, :], w3_sb[:],
                            y2t[:, h * NB:(h + 1) * NB],
                            start=False, stop=(tl == 3),
                            tile_position=(0, 32 * tl))
                    ex = ep.tile([EP, NB], F16, tag="ex")
                    nc.scalar.activation(ex[:], pg[:], AF.Exp)
                    ps_l = ps4.tile([EP, NB], F32, tag="lse", name="ps_l")
                    nc.tensor.matmul(ps_l[:], obd_sb[:], ex[:], start=True,
                                     stop=True)
                    ls = ep.tile([EP, NB], F32, tag="ls")
                    nc.scalar.activation(ls[:], ps_l[:], AF.Ln)
                    o = ep.tile([EP, NB], F16, tag="o")
                    nc.vector.tensor_tensor(o[:], pg[:], ls[:],
                                            op=ALU.subtract)
                    for tl in range(GRP):
                        t = GRP * g + tl
                        ring = nc.sync if tl % 2 == 0 else nc.scalar
                        ring.dma_start(
                            out_d[:, t * NB:(t + 1) * NB],
                            o[32 * tl:32 * tl + C, :])

    nc.compile()
    return nc


_CACHE = {}


def _prepare(x, W1, b1, W2, b2, W3, b3, mask1, mask2, mask3):
    B, D1 = x.shape
    H = W2.shape[0]
    C = W3.shape[0]
    assert B % N_CORES == 0
    Bc = B // N_CORES

    S, R_list = _decompose_mask1(np.asarray(mask1))
    R_lens = [len(r) for r in R_list]
    n_blk = len(R_list)
    blk_w = [min(OT, D1 - j * OT) for j in range(n_blk)]
    P_pad = max(R_lens + [1])
    nS = len(S)
    n_sc = (nS + 127) // 128
    sc_w = -(-nS // n_sc)
    n_lane = n_sc + n_blk
    EP = 32 * (GRP - 1) + C

    Wm1 = (np.asarray(W1) * np.asarray(mask1)).astype(np.float32)
    Wm2 = (np.asarray(W2) * np.asarray(mask2)).astype(np.float32)
    Wm3 = (np.asarray(W3) * np.asarray(mask3)).astype(np.float32)

    c16 = lambda a: np.asarray(a, dtype=np.float16)
    c8 = lambda a: np.asarray(a, dtype=NP8)

    # ---- weights ----
    ws8 = np.zeros((sc_w, n_sc, D1), np.float32)
    for c in range(n_sc):
        rows = S[c * sc_w:(c + 1) * sc_w]
        ws8[:len(rows), c, :] = Wm1[:, rows].T * WSCALE
    wb8 = np.zeros((P_pad, D1), np.float32)
    for j, R in enumerate(R_list):
        wb8[:len(R), j * OT:j * OT + blk_w[j]] = \
            Wm1[j * OT:j * OT + blk_w[j], R].T * WSCALE
    n_pr = n_blk // 2
    lw = blk_w[-1] if n_blk % 2 else 0
    w2 = np.zeros((OT, 2, n_pr * H), np.float32)
    for q in range(n_pr):
        for o in range(2):
            j = 2 * q + o
            w2[:blk_w[j], o, q * H:(q + 1) * H] = \
                Wm2[:, j * OT:j * OT + blk_w[j]].T * W2SCALE
    w26 = np.zeros((max(lw, 1), H), np.float32)
    if lw:
        w26[:lw, :] = Wm2[:, (n_blk - 1) * OT:].T * W2SCALE
    w3 = np.ascontiguousarray(Wm3.T)                      # [H, C]
    b1p = np.zeros((OT, n_blk), np.float32)
    for j in range(n_blk):
        b1p[:blk_w[j], j] = WSCALE * \
            np.asarray(b1, np.float32)[j * OT:j * OT + blk_w[j]]
    b2p = np.asarray(b2, np.float32).reshape(H, 1)
    b3r = np.zeros((1, EP), np.float32)
    obd = np.zeros((EP, EP), np.float32)
    for tl in range(GRP):
        b3r[0, 32 * tl:32 * tl + C] = np.asarray(b3, np.float32)
        for m in range(32 * tl, min(32 * tl + 32, EP)):
            obd[32 * tl:32 * tl + C, m] = 1.0
    onec = np.ones((1, NB), np.float32)

    # ---- x slabs: one array per superchunk [NC, P_pad, n_lane, S_w] ----
    xT = np.asarray(x, np.float32).T                      # [D1, B] view
    xarrs = [np.zeros((N_CORES, P_pad, n_lane, S_w), NP8) for S_w in SCS]

    def fill_lane(lane, rows):
        data = c8(xT[rows]).reshape(len(rows), N_CORES, Bc)
        start = 0
        for s, S_w in enumerate(SCS):
            xarrs[s][:, :len(rows), lane, :] = \
                data[:, :, start:start + S_w].transpose(1, 0, 2)
            start += S_w

    for c in range(n_sc):
        fill_lane(c, S[c * sc_w:(c + 1) * sc_w])
    for j, R in enumerate(R_list):
        fill_lane(n_sc + j, R)

    meta = dict(nS=nS, R_lens=R_lens, P_pad=P_pad, Bc=Bc, D1=D1, H=H, C=C)
    key = (B, D1, H, C, nS, tuple(R_lens))
    if key not in _CACHE:
        _CACHE[key] = _build_program(meta)
    nc = _CACHE[key]

    in_maps = []
    for cidx in range(N_CORES):
        m = {f"x{s}": xarrs[s][cidx] for s in range(len(SCS))}
        m.update({
            "ws8": c8(ws8), "wb8": c8(wb8), "w2": c8(w2), "w3": c16(w3),
            "b1": b1p, "b2": b2p,
            "b3r": c16(b3r), "onec": c16(onec), "obd": c16(obd),
        })
        if lw:
            m["w26"] = c8(w26)
        in_maps.append(m)
    return nc, in_maps, meta


def _assemble(results, meta):
    outs = [np.ascontiguousarray(results[c]["out"].T).astype(np.float32)
            for c in range(N_CORES)]
    return np.concatenate(outs, axis=0)


def kernel(**inputs):
    nc, in_maps, meta = _prepare(**inputs)
    res = bass_utils.run_bass_kernel_spmd(nc, in_maps,
                                          core_ids=list(range(N_CORES)))
    return _assemble(res.results, meta)


def kernel_traced(tmpdir=None, **inputs):
    """Same as kernel() but with NTFF profiling; returns (output, results)."""
    nc, in_maps, meta = _prepare(**inputs)
    res = bass_utils.run_bass_kernel_spmd(nc, in_maps,
                                          core_ids=list(range(N_CORES)),
                                          trace=True, tmpdir=tmpdir)
    return _assemble(res.results, meta), res


# revision 15
# speedup vs baseline: 1.5138x; 1.0431x over previous
"""Trainium2 Bass kernel for the ButterflyMlp problem.

Computes log_softmax(L3(relu(L2(relu(L1(x)))))) where each Li is a masked
linear layer (butterfly sparsity: global column stripes + a diagonal band),
batch 65536, data-parallel over 8 NeuronCores (8192 rows/core).

Strategy (per core, feature-major throughout):
  - Masks are pre-applied to weights on host. Layer-1 exploits the butterfly
    structure: the stripe columns (mask true for every output row) form a
    dense [|S|, 784] GEMM shared by all outputs, and the per-output-block
    band adds one narrow [|R_j|<=128, 128] GEMM per 128-row output block.
  - Layers 1 and 2 run in fp8-e4m3 (x, masked W1, y1, and W2 quantized;
    W1 scaled by 16 and W2 by 8 to stay in fp8 normal range, the combined
    x128 undone for free by the ACT scale on the y2 eviction — exact linear
    rescale, no extra ops). The 204 stripe columns contract in ONE matmul
    per output block via DoubleRow perf mode (2 fp8 k-rows per PE cell),
    halving the stripe matmul count, and layer-2's 128-wide k-chunks pair
    up the same way (4 matmuls instead of 7). Host-measured end-to-end
    error of the full quantization is ~1.1e-2 vs the 2e-2 budget. fp8 also
    halves the x DMA stream (8.1 MB/core), which otherwise outruns SWDGE
    and stalls the PE.
  - PSUM tiles are 2 banks wide ([*, 1024]); each relu+bias eviction covers
    a 1024-column chunk pair, halving ACT/DVE per-op overhead.
  - log_softmax is batched 4 chunks at a time: the four [10, 512] layer-3
    GEMMs are col-tiled (tile_position=(0, 32*t)) into one PSUM bank at
    partition offsets 0/32/64/96 and run concurrently; b3 is accumulated
    into the same bank by a K=1 ones-matmul; one block-diagonal ones-matmul
    computes all four groups' sum(exp) at once. Exp/Ln then process
    [106, 512] (4 chunks) per op instead of [10, 512] per chunk, and the
    final subtract reads PSUM directly (no y3 eviction op at all).
  - All x data for a superchunk moves in ONE SWDGE transfer; the first two
    superchunks are small (512 cols) so the first matmul fires early.
"""
import sys
sys.path.insert(0, "/opt/trn_rl_repo")
import numpy as np
import ml_dtypes

import concourse.bass as bass
import concourse.bacc as bacc
import concourse.mybir as mybir
import concourse.tile as tile
from concourse import bass_utils

F32 = mybir.dt.float32
F16 = mybir.dt.float16
F8 = mybir.dt.float8e4
NP8 = ml_dtypes.float8_e4m3
AF = mybir.ActivationFunctionType
ALU = mybir.AluOpType
DR = mybir.MatmulPerfMode.DoubleRow

# All activation functions this kernel uses live together in the
# natural_log_exp_and_others table set, but the greedy per-function set
# chooser picks exp_and_others for Exp and natural_log* for Ln, reloading
# ACT tables twice per chunk (~1.3us each). Restrict every other set's
# advertised contents so the chooser lands on the one set that covers
# everything and emits a single load. Set ids stay valid: the dict keys
# and order are unchanged.
_PIN_SET = "natural_log_exp_and_others"
_orig_gat = bacc.get_activation_tables


def _pinned_gat(arch):
    tabs = _orig_gat(arch)
    need = {AF.Relu, AF.Identity, AF.Exp, AF.Ln, AF.Copy}
    if _PIN_SET in tabs and need <= tabs[_PIN_SET]:
        for name in tabs:
            if name != _PIN_SET:
                tabs[name] = tabs[name] - need
    return tabs


bacc.get_activation_tables = _pinned_gat

N_CORES = 8
NB = 512          # batch columns per matmul (one PSUM bank of fp32)
OT = 128          # layer-1 output block width (6x128 + 1x16)
SCS = [512, 512, 1024, 2048, 2048, 2048]   # superchunk widths (sum = Bc)
GRP = 4           # NB-chunks per log_softmax group (4 col-tile slots)
WSCALE = 16.0     # fp8 W1 pre-scale
W2SCALE = 8.0     # fp8 W2 pre-scale (y2 eviction divides by 16*8)


def _decompose_mask1(mask1):
    """Split the butterfly mask into stripe columns S (true for every row)
    and per-output-block residual columns R_j (blocks of OT rows)."""
    D_out, D_in = mask1.shape
    S = np.where(mask1.all(axis=0))[0]
    n_blk = (D_out + OT - 1) // OT
    stripe_set = np.zeros(D_in, dtype=bool)
    stripe_set[S] = True
    R_list = []
    for j in range(n_blk):
        blk = mask1[j * OT:(j + 1) * OT]
        cols = np.where(blk.any(axis=0) & ~stripe_set)[0]
        assert len(cols) <= 128, f"band block {j} has {len(cols)} cols"
        R_list.append(cols)
    return S, R_list


def _build_program(meta):
    nS, R_lens = meta["nS"], meta["R_lens"]
    P_pad = meta["P_pad"]
    Bc = meta["Bc"]
    D1, H, C = meta["D1"], meta["H"], meta["C"]
    n_blk = len(R_lens)
    blk_w = [min(OT, D1 - j * OT) for j in range(n_blk)]
    n_sc = (nS + 127) // 128              # stripe K-chunks
    sc_w = -(-nS // n_sc)                 # stripe chunk width (padded)
    use_dr = (n_sc == 2)                  # DoubleRow wants exactly 2 chunks
    n_lane = n_sc + n_blk                 # x lanes per superchunk slab
    n_ch = Bc // NB                       # NB chunks per core
    assert sum(SCS) == Bc
    loc = []                              # chunk -> (superchunk, local col)
    for s, S_w in enumerate(SCS):
        for co in range(0, S_w, NB):
            loc.append((s, co))
    assert len(loc) == n_ch
    EP = 32 * (GRP - 1) + C               # epilogue partition span (106)
    n_pr = n_blk // 2                     # layer-2 DoubleRow k-chunk pairs
    lw = blk_w[-1] if n_blk % 2 else 0    # leftover block width
    assert all(blk_w[2 * q] == OT and blk_w[2 * q + 1] == OT
               for q in range(n_pr)), "layer-2 DR pairs need full blocks"

    nc = bacc.Bacc("TRN2", target_bir_lowering=False, debug=False,
                   enable_asserts=False, num_devices=N_CORES)

    x_d = [nc.dram_tensor(f"x{s}", [P_pad, n_lane, S_w], F8,
                          kind="ExternalInput").ap()
           for s, S_w in enumerate(SCS)]
    ws8_d = nc.dram_tensor("ws8", [sc_w, n_sc, D1], F8,
                           kind="ExternalInput").ap()
    wb8_d = nc.dram_tensor("wb8", [P_pad, D1], F8,
                           kind="ExternalInput").ap()
    w2_d = nc.dram_tensor("w2", [OT, 2, n_pr * H], F8,
                          kind="ExternalInput").ap()
    w26_d = (nc.dram_tensor("w26", [lw, H], F8, kind="ExternalInput").ap()
             if lw else None)
    w3_d = nc.dram_tensor("w3", [H, C], F16, kind="ExternalInput").ap()
    b1_d = nc.dram_tensor("b1", [OT, n_blk], F32, kind="ExternalInput").ap()
    b2_d = nc.dram_tensor("b2", [H, 1], F32, kind="ExternalInput").ap()
    b3r_d = nc.dram_tensor("b3r", [1, EP], F16, kind="ExternalInput").ap()
    onec_d = nc.dram_tensor("onec", [1, NB], F16, kind="ExternalInput").ap()
    obd_d = nc.dram_tensor("obd", [EP, EP], F16, kind="ExternalInput").ap()
    out_d = nc.dram_tensor("out", [C, Bc], F16, kind="ExternalOutput").ap()

    with tile.TileContext(nc) as tc:
        with tc.tile_pool(name="wp", bufs=1) as wp, \
             tc.tile_pool(name="xp", bufs=len(SCS)) as xp, \
             tc.tile_pool(name="hp", bufs=2) as hp, \
             tc.tile_pool(name="yp", bufs=3) as yp, \
             tc.tile_pool(name="ep", bufs=2) as ep, \
             tc.tile_pool(name="ps1", bufs=2, space="PSUM") as ps1, \
             tc.tile_pool(name="ps2", bufs=1, space="PSUM") as ps2, \
             tc.tile_pool(name="ps3", bufs=1, space="PSUM") as ps3, \
             tc.tile_pool(name="ps4", bufs=1, space="PSUM") as ps4:

            # ---- resident weights first on SWDGE (small, ~0.3 MB; the
            # HWDGE rings emit per-partition ~300 B packets at ~2.6 GB/s
            # for these strided slabs — measured 10-25 us there)
            ws8_sb = wp.tile([sc_w, n_sc, D1], F8)
            nc.gpsimd.dma_start(ws8_sb[:], ws8_d[:])
            wb8_sb = wp.tile([P_pad, D1], F8)
            nc.gpsimd.dma_start(wb8_sb[:], wb8_d[:])
            w2_sb = wp.tile([OT, 2, n_pr * H], F8)
            nc.gpsimd.dma_start(w2_sb[:], w2_d[:])
            if lw:
                w26_sb = wp.tile([lw, H], F8)
                nc.gpsimd.dma_start(w26_sb[:], w26_d[:])

            # ---- every superchunk's x load: ONE packed SWDGE DMA each,
            # all resident simultaneously (73.7 KB/partition total) so the
            # stream never stalls on a tile-slot release.
            x_tiles = []
            for s, S_w in enumerate(SCS):
                xt = xp.tile([P_pad, n_lane, S_w], F8, name="xt", tag="xt")
                nc.gpsimd.dma_start(xt[:], x_d[s][:])
                x_tiles.append(xt)

            w3_sb = wp.tile([H, C], F16)
            nc.sync.dma_start(w3_sb[:], w3_d[:])
            b1_sb = wp.tile([OT, n_blk], F32)
            nc.sync.dma_start(b1_sb[:], b1_d[:])
            b2_sb = wp.tile([H, 1], F32)
            nc.sync.dma_start(b2_sb[:], b2_d[:])
            b3r_sb = wp.tile([1, EP], F16)
            nc.sync.dma_start(b3r_sb[:], b3r_d[:])
            onec_sb = wp.tile([1, NB], F16)
            nc.sync.dma_start(onec_sb[:], onec_d[:])
            obd_sb = wp.tile([EP, EP], F16)
            nc.sync.dma_start(obd_sb[:], obd_d[:])

            y2_tiles = []
            for p in range(n_ch // 2):
                t0, t1 = 2 * p, 2 * p + 1
                halves = [loc[t0], loc[t1]]

                # ---- layer 1: 7 output blocks, PSUM tile spans the pair;
                # evictions write fp8 into DoubleRow-paired k-chunk tiles
                y1p = [hp.tile([OT, 2, 2 * NB], F8, name=f"y1p_{q}",
                               tag=f"y1p{q}") for q in range(n_pr)]
                y1l = (hp.tile([lw, 2 * NB], F8, name="y1l", tag="y1l")
                       if lw else None)
                for j in range(n_blk):
                    wj = blk_w[j]
                    pt = ps1.tile([wj, 2 * NB], F32, tag="l1", name="p1")
                    for h, (s, co) in enumerate(halves):
                        xt = x_tiles[s]
                        if use_dr:
                            nc.tensor.matmul(
                                pt[:, h * NB:(h + 1) * NB],
                                ws8_sb[:, :, j * OT:j * OT + wj],
                                xt[:sc_w, 0:n_sc, co:co + NB],
                                start=True, stop=False, perf_mode=DR)
                        else:
                            for c in range(n_sc):
                                kw = nS - c * sc_w if c == n_sc - 1 else sc_w
                                nc.tensor.matmul(
                                    pt[:, h * NB:(h + 1) * NB],
                                    ws8_sb[:kw, c, j * OT:j * OT + wj],
                                    xt[:kw, c:c + 1, co:co + NB],
                                    start=(c == 0), stop=False)
                        nc.tensor.matmul(
                            pt[:, h * NB:(h + 1) * NB],
                            wb8_sb[:R_lens[j], j * OT:j * OT + wj],
                            xt[:R_lens[j], n_sc + j:n_sc + j + 1, co:co + NB],
                            start=False, stop=True)
                    dst = (y1p[j // 2][:wj, j % 2, :] if j < 2 * n_pr
                           else y1l[:])
                    if j % 2 == 0:
                        nc.vector.tensor_scalar(dst, pt[:],
                                                b1_sb[:wj, j:j + 1], 0.0,
                                                op0=ALU.add, op1=ALU.max)
                    else:
                        nc.scalar.activation(dst, pt[:], AF.Relu,
                                             bias=b1_sb[:wj, j:j + 1])

                # ---- layer 2 (fp8 DoubleRow over k-chunk pairs) ----
                p2 = ps2.tile([H, 2 * NB], F32, tag="l2", name="p2")
                for q in range(n_pr):
                    for h in range(2):
                        nc.tensor.matmul(
                            p2[:, h * NB:(h + 1) * NB],
                            w2_sb[:, :, q * H:(q + 1) * H],
                            y1p[q][:, :, h * NB:(h + 1) * NB],
                            start=(q == 0), stop=(q == n_pr - 1 and not lw),
                            perf_mode=DR)
                if lw:
                    for h in range(2):
                        nc.tensor.matmul(
                            p2[:, h * NB:(h + 1) * NB], w26_sb[:],
                            y1l[:, h * NB:(h + 1) * NB],
                            start=False, stop=True)
                # undo the fp8 weight pre-scales (x16 from W1, x8 from W2)
                y2 = yp.tile([H, 2 * NB], F16, tag="y2")
                nc.scalar.activation(y2[:], p2[:], AF.Relu,
                                     bias=b2_sb[:, 0:1],
                                     scale=1.0 / (WSCALE * W2SCALE))
                y2_tiles.append(y2)

                # ---- every 2 pairs: batched layer 3 + log_softmax for 4
                # chunks, col-tiled into one PSUM bank at partition
                # offsets 0/32/64/96.
                if p % 2 == 1:
                    g = p // 2
                    pg = ps3.tile([EP, NB], F32, tag="l3", name="pg")
                    # b3 broadcast into all slots via a K=1 ones-matmul
                    # (start=True clears the bank; col MMs accumulate).
                    nc.tensor.matmul(pg[:], b3r_sb[0:1, :], onec_sb[0:1, :],
                                     start=True, stop=False)
                    srcs = [(y2_tiles[p - 1], 0), (y2_tiles[p - 1], 1),
                            (y2_tiles[p], 0), (y2_tiles[p], 1)]
                    for tl, (y2t, h) in enumerate(srcs):
                        nc.tensor.matmul(
                            pg[32 * tl:32 * tl + C, :], w3_sb[:],
                            y2t[:, h * NB:(h + 1) * NB],
                            start=False, stop=(tl == 3),
                            tile_position=(0, 32 * tl))
                    ex = ep.tile([EP, NB], F16, tag="ex")
                    nc.scalar.activation(ex[:], pg[:], AF.Exp)
                    ps_l = ps4.tile([EP, NB], F32, tag="lse", name="ps_l")
                    nc.tensor.matmul(ps_l[:], obd_sb[:], ex[:], start=True,
                                     stop=True)
                    ls = ep.tile([EP, NB], F32, tag="ls")
                    nc.scalar.activation(ls[:], ps_l[:], AF.Ln)
                    o = ep.tile([EP, NB], F16, tag="o")
                    nc.vector.tensor_tensor(o[:], pg[:], ls[:],
                                            op=ALU.subtract)
                    for tl in range(GRP):
                        t = GRP * g + tl
                        ring = nc.sync if tl % 2 == 0 else nc.scalar
                        ring.dma_start(
                            out_d[:, t * NB:(t + 1) * NB],
                            o[32 * tl:32 * tl + C, :])

    nc.compile()
    return nc


_CACHE = {}


def _prepare(x, W1, b1, W2, b2, W3, b3, mask1, mask2, mask3):
    B, D1 = x.shape
    H = W2.shape[0]
    C = W3.shape[0]
    assert B % N_CORES == 0
    Bc = B // N_CORES

    S, R_list = _decompose_mask1(np.asarray(mask1))
    R_lens = [len(r) for r in R_list]
    n_blk = len(R_list)
    blk_w = [min(OT, D1 - j * OT) for j in range(n_blk)]
    P_pad = max(R_lens + [1])
    nS = len(S)
    n_sc = (nS + 127) // 128
    sc_w = -(-nS // n_sc)
    n_lane = n_sc + n_blk
    EP = 32 * (GRP - 1) + C

    Wm1 = (np.asarray(W1) * np.asarray(mask1)).astype(np.float32)
    Wm2 = (np.asarray(W2) * np.asarray(mask2)).astype(np.float32)
    Wm3 = (np.asarray(W3) * np.asarray(mask3)).astype(np.float32)

    c16 = lambda a: np.asarray(a, dtype=np.float16)
    c8 = lambda a: np.asarray(a, dtype=NP8)

    # ---- weights ----
    ws8 = np.zeros((sc_w, n_sc, D1), np.float32)
    for c in range(n_sc):
        rows = S[c * sc_w:(c + 1) * sc_w]
        ws8[:len(rows), c, :] = Wm1[:, rows].T * WSCALE
    wb8 = np.zeros((P_pad, D1), np.float32)
    for j, R in enumerate(R_list):
        wb8[:len(R), j * OT:j * OT + blk_w[j]] = \
            Wm1[j * OT:j * OT + blk_w[j], R].T * WSCALE
    n_pr = n_blk // 2
    lw = blk_w[-1] if n_blk % 2 else 0
    w2 = np.zeros((OT, 2, n_pr * H), np.float32)
    for q in range(n_pr):
        for o in range(2):
            j = 2 * q + o
            w2[:blk_w[j], o, q * H:(q + 1) * H] = \
                Wm2[:, j * OT:j * OT + blk_w[j]].T * W2SCALE
    w26 = np.zeros((max(lw, 1), H), np.float32)
    if lw:
        w26[:lw, :] = Wm2[:, (n_blk - 1) * OT:].T * W2SCALE
    w3 = np.ascontiguousarray(Wm3.T)                      # [H, C]
    b1p = np.zeros((OT, n_blk), np.float32)
    for j in range(n_blk):
        b1p[:blk_w[j], j] = WSCALE * \
            np.asarray(b1, np.float32)[j * OT:j * OT + blk_w[j]]
    b2p = np.asarray(b2, np.float32).reshape(H, 1)
    b3r = np.zeros((1, EP), np.float32)
    obd = np.zeros((EP, EP), np.float32)
    for tl in range(GRP):
        b3r[0, 32 * tl:32 * tl + C] = np.asarray(b3, np.float32)
        for m in range(32 * tl, min(32 * tl + 32, EP)):
            obd[32 * tl:32 * tl + C, m] = 1.0
    onec = np.ones((1, NB), np.float32)

    # ---- x slabs: one array per superchunk [NC, P_pad, n_lane, S_w] ----
    xT = np.asarray(x, np.float32).T                      # [D1, B] view
    xarrs = [np.zeros((N_CORES, P_pad, n_lane, S_w), NP8) for S_w in SCS]

    def fill_lane(lane, rows):
        data = c8(xT[rows]).reshape(len(rows), N_CORES, Bc)
        start = 0
        for s, S_w in enumerate(SCS):
            xarrs[s][:, :len(rows), lane, :] = \
                data[:, :, start:start + S_w].transpose(1, 0, 2)
            start += S_w

    for c in range(n_sc):
        fill_lane(c, S[c * sc_w:(c + 1) * sc_w])
    for j, R in enumerate(R_list):
        fill_lane(n_sc + j, R)

    meta = dict(nS=nS, R_lens=R_lens, P_pad=P_pad, Bc=Bc, D1=D1, H=H, C=C)
    key = (B, D1, H, C, nS, tuple(R_lens))
    if key not in _CACHE:
        _CACHE[key] = _build_program(meta)
    nc = _CACHE[key]

    in_maps = []
    for cidx in range(N_CORES):
        m = {f"x{s}": xarrs[s][cidx] for s in range(len(SCS))}
        m.update({
            "ws8": c8(ws8), "wb8": c8(wb8), "w2": c8(w2), "w3": c16(w3),
            "b1": b1p, "b2": b2p,
            "b3r": c16(b3r), "onec": c16(onec), "obd": c16(obd),
        })
        if lw:
            m["w26"] = c8(w26)
        in_maps.append(m)
    return nc, in_maps, meta


def _assemble(results, meta):
    outs = [np.ascontiguousarray(results[c]["out"].T).astype(np.float32)
            for c in range(N_CORES)]
    return np.concatenate(outs, axis=0)


def kernel(**inputs):
    nc, in_maps, meta = _prepare(**inputs)
    res = bass_utils.run_bass_kernel_spmd(nc, in_maps,
                                          core_ids=list(range(N_CORES)))
    return _assemble(res.results, meta)


def kernel_traced(tmpdir=None, **inputs):
    """Same as kernel() but with NTFF profiling; returns (output, results)."""
    nc, in_maps, meta = _prepare(**inputs)
    res = bass_utils.run_bass_kernel_spmd(nc, in_maps,
                                          core_ids=list(range(N_CORES)),
                                          trace=True, tmpdir=tmpdir)
    return _assemble(res.results, meta), res


# revision 18
# speedup vs baseline: 1.5495x; 1.0236x over previous
"""Trainium2 Bass kernel for the ButterflyMlp problem.

Computes log_softmax(L3(relu(L2(relu(L1(x)))))) where each Li is a masked
linear layer (butterfly sparsity: global column stripes + a diagonal band),
batch 65536, data-parallel over 8 NeuronCores (8192 rows/core).

Strategy (per core, feature-major throughout):
  - Masks are pre-applied to weights on host. Layer-1 exploits the butterfly
    structure: the stripe columns (mask true for every output row) form a
    dense [|S|, 784] GEMM shared by all outputs, and the per-output-block
    band adds one narrow [|R_j|<=128, 128] GEMM per 128-row output block.
  - Layers 1 and 2 run in fp8-e4m3 (x, masked W1, y1, and W2 quantized;
    W1 scaled by 16 and W2 by 8 to stay in fp8 normal range, the combined
    x128 undone for free by the ACT scale on the y2 eviction — exact linear
    rescale, no extra ops). The 204 stripe columns contract in ONE matmul
    per output block via DoubleRow perf mode (2 fp8 k-rows per PE cell),
    halving the stripe matmul count, and layer-2's 128-wide k-chunks pair
    up the same way (4 matmuls instead of 7). Host-measured end-to-end
    error of the full quantization is ~1.1e-2 vs the 2e-2 budget. fp8 also
    halves the x DMA stream (8.1 MB/core), which otherwise outruns SWDGE
    and stalls the PE.
  - PSUM tiles are 2 banks wide ([*, 1024]); each relu+bias eviction covers
    a 1024-column chunk pair, halving ACT/DVE per-op overhead.
  - log_softmax is batched 4 chunks at a time: the four [10, 512] layer-3
    GEMMs are col-tiled (tile_position=(0, 32*t)) into one PSUM bank at
    partition offsets 0/32/64/96 and run concurrently; b3 is accumulated
    into the same bank by a K=1 ones-matmul; one block-diagonal ones-matmul
    computes all four groups' sum(exp) at once. Exp/Ln then process
    [106, 512] (4 chunks) per op instead of [10, 512] per chunk, and the
    final subtract reads PSUM directly (no y3 eviction op at all).
  - All x data for a superchunk moves in ONE SWDGE transfer; the first two
    superchunks are small (512 cols) so the first matmul fires early.
"""
import sys
sys.path.insert(0, "/opt/trn_rl_repo")
import numpy as np
import ml_dtypes

import concourse.bass as bass
import concourse.bacc as bacc
import concourse.mybir as mybir
import concourse.tile as tile
from concourse import bass_utils

F32 = mybir.dt.float32
F16 = mybir.dt.float16
F8 = mybir.dt.float8e4
NP8 = ml_dtypes.float8_e4m3
AF = mybir.ActivationFunctionType
ALU = mybir.AluOpType
DR = mybir.MatmulPerfMode.DoubleRow

# All activation functions this kernel uses live together in the
# natural_log_exp_and_others table set, but the greedy per-function set
# chooser picks exp_and_others for Exp and natural_log* for Ln, reloading
# ACT tables twice per chunk (~1.3us each). Restrict every other set's
# advertised contents so the chooser lands on the one set that covers
# everything and emits a single load. Set ids stay valid: the dict keys
# and order are unchanged.
_PIN_SET = "natural_log_exp_and_others"
_orig_gat = bacc.get_activation_tables


def _pinned_gat(arch):
    tabs = _orig_gat(arch)
    need = {AF.Relu, AF.Identity, AF.Exp, AF.Ln, AF.Copy}
    if _PIN_SET in tabs and need <= tabs[_PIN_SET]:
        for name in tabs:
            if name != _PIN_SET:
                tabs[name] = tabs[name] - need
    return tabs


bacc.get_activation_tables = _pinned_gat

N_CORES = 8
NB = 512          # batch columns per matmul (one PSUM bank of fp32)
OT = 128          # layer-1 output block width (6x128 + 1x16)
SCS = [512, 512, 1024, 2048, 2048, 2048]   # superchunk widths (sum = Bc)
GRP = 4           # NB-chunks per log_softmax group (4 col-tile slots)
WSCALE = 16.0     # fp8 W1 pre-scale
W2SCALE = 8.0     # fp8 W2 pre-scale (y2 eviction divides by 16*8)


def _decompose_mask1(mask1):
    """Split the butterfly mask into stripe columns S (true for every row)
    and per-output-block residual columns R_j (blocks of OT rows)."""
    D_out, D_in = mask1.shape
    S = np.where(mask1.all(axis=0))[0]
    n_blk = (D_out + OT - 1) // OT
    stripe_set = np.zeros(D_in, dtype=bool)
    stripe_set[S] = True
    R_list = []
    for j in range(n_blk):
        blk = mask1[j * OT:(j + 1) * OT]
        cols = np.where(blk.any(axis=0) & ~stripe_set)[0]
        assert len(cols) <= 128, f"band block {j} has {len(cols)} cols"
        R_list.append(cols)
    return S, R_list


def _build_program(meta):
    nS, R_lens = meta["nS"], meta["R_lens"]
    P_pad = meta["P_pad"]
    Bc = meta["Bc"]
    D1, H, C = meta["D1"], meta["H"], meta["C"]
    n_blk = len(R_lens)
    blk_w = [min(OT, D1 - j * OT) for j in range(n_blk)]
    n_sc = (nS + 127) // 128              # stripe K-chunks
    sc_w = -(-nS // n_sc)                 # stripe chunk width (padded)
    use_dr = (n_sc == 2)                  # DoubleRow wants exactly 2 chunks
    n_lane = n_sc + n_blk                 # x lanes per superchunk slab
    n_ch = Bc // NB                       # NB chunks per core
    assert sum(SCS) == Bc
    loc = []                              # chunk -> (superchunk, local col)
    for s, S_w in enumerate(SCS):
        for co in range(0, S_w, NB):
            loc.append((s, co))
    assert len(loc) == n_ch
    EP = 32 * (GRP - 1) + C               # epilogue partition span (106)
    n_pr = n_blk // 2                     # layer-2 DoubleRow k-chunk pairs
    lw = blk_w[-1] if n_blk % 2 else 0    # leftover block width
    assert all(blk_w[2 * q] == OT and blk_w[2 * q + 1] == OT
               for q in range(n_pr)), "layer-2 DR pairs need full blocks"

    nc = bacc.Bacc("TRN2", target_bir_lowering=False, debug=False,
                   enable_asserts=False, num_devices=N_CORES)

    x_d = [nc.dram_tensor(f"x{s}", [P_pad, n_lane, S_w], F8,
                          kind="ExternalInput").ap()
           for s, S_w in enumerate(SCS)]
    ws8_d = nc.dram_tensor("ws8", [sc_w, n_sc, D1], F8,
                           kind="ExternalInput").ap()
    wb8_d = nc.dram_tensor("wb8", [P_pad, D1], F8,
                           kind="ExternalInput").ap()
    w2_d = nc.dram_tensor("w2", [OT, 2, n_pr * H], F8,
                          kind="ExternalInput").ap()
    w26_d = (nc.dram_tensor("w26", [lw, H], F8, kind="ExternalInput").ap()
             if lw else None)
    w3_d = nc.dram_tensor("w3", [H, C], F16, kind="ExternalInput").ap()
    b1_d = nc.dram_tensor("b1", [OT, n_blk], F32, kind="ExternalInput").ap()
    b2_d = nc.dram_tensor("b2", [H, 1], F32, kind="ExternalInput").ap()
    b3r_d = nc.dram_tensor("b3r", [1, EP], F16, kind="ExternalInput").ap()
    onec_d = nc.dram_tensor("onec", [1, NB], F16, kind="ExternalInput").ap()
    obd_d = nc.dram_tensor("obd", [EP, EP], F16, kind="ExternalInput").ap()
    out_d = nc.dram_tensor("out", [C, Bc], F16, kind="ExternalOutput").ap()

    with tile.TileContext(nc) as tc:
        with tc.tile_pool(name="wp", bufs=1) as wp, \
             tc.tile_pool(name="xp", bufs=len(SCS)) as xp, \
             tc.tile_pool(name="hp", bufs=2) as hp, \
             tc.tile_pool(name="yp", bufs=4) as yp, \
             tc.tile_pool(name="ep", bufs=2) as ep, \
             tc.tile_pool(name="ps1", bufs=2, space="PSUM") as ps1, \
             tc.tile_pool(name="ps2", bufs=1, space="PSUM") as ps2, \
             tc.tile_pool(name="ps3", bufs=1, space="PSUM") as ps3, \
             tc.tile_pool(name="ps4", bufs=1, space="PSUM") as ps4:

            # ---- resident weights first on SWDGE (small, ~0.3 MB; the
            # HWDGE rings emit per-partition ~300 B packets at ~2.6 GB/s
            # for these strided slabs — measured 10-25 us there)
            ws8_sb = wp.tile([sc_w, n_sc, D1], F8)
            nc.gpsimd.dma_start(ws8_sb[:], ws8_d[:])
            wb8_sb = wp.tile([P_pad, D1], F8)
            nc.gpsimd.dma_start(wb8_sb[:], wb8_d[:])
            w2_sb = wp.tile([OT, 2, n_pr * H], F8)
            nc.gpsimd.dma_start(w2_sb[:], w2_d[:])
            if lw:
                w26_sb = wp.tile([lw, H], F8)
                nc.gpsimd.dma_start(w26_sb[:], w26_d[:])

            # ---- every superchunk's x load: ONE packed SWDGE DMA each,
            # all resident simultaneously (73.7 KB/partition total) so the
            # stream never stalls on a tile-slot release.
            x_tiles = []
            for s, S_w in enumerate(SCS):
                xt = xp.tile([P_pad, n_lane, S_w], F8, name="xt", tag="xt")
                nc.gpsimd.dma_start(xt[:], x_d[s][:])
                x_tiles.append(xt)

            w3_sb = wp.tile([H, C], F16)
            nc.sync.dma_start(w3_sb[:], w3_d[:])
            b1_sb = wp.tile([OT, n_blk], F32)
            nc.sync.dma_start(b1_sb[:], b1_d[:])
            b2_sb = wp.tile([H, 1], F32)
            nc.sync.dma_start(b2_sb[:], b2_d[:])
            b3r_sb = wp.tile([1, EP], F16)
            nc.sync.dma_start(b3r_sb[:], b3r_d[:])
            onec_sb = wp.tile([1, NB], F16)
            nc.sync.dma_start(onec_sb[:], onec_d[:])
            obd_sb = wp.tile([EP, EP], F16)
            nc.sync.dma_start(obd_sb[:], obd_d[:])

            y2_tiles = []
            for p in range(n_ch // 2):
                t0, t1 = 2 * p, 2 * p + 1
                halves = [loc[t0], loc[t1]]

                # ---- layer 1: 7 output blocks, PSUM tile spans the pair;
                # evictions write fp8 into DoubleRow-paired k-chunk tiles
                y1p = [hp.tile([OT, 2, 2 * NB], F8, name=f"y1p_{q}",
                               tag=f"y1p{q}") for q in range(n_pr)]
                y1l = (hp.tile([lw, 2 * NB], F8, name="y1l", tag="y1l")
                       if lw else None)
                for j in range(n_blk):
                    wj = blk_w[j]
                    pt = ps1.tile([wj, 2 * NB], F32, tag="l1", name="p1")
                    for h, (s, co) in enumerate(halves):
                        xt = x_tiles[s]
                        if use_dr:
                            nc.tensor.matmul(
                                pt[:, h * NB:(h + 1) * NB],
                                ws8_sb[:, :, j * OT:j * OT + wj],
                                xt[:sc_w, 0:n_sc, co:co + NB],
                                start=True, stop=False, perf_mode=DR)
                        else:
                            for c in range(n_sc):
                                kw = nS - c * sc_w if c == n_sc - 1 else sc_w
                                nc.tensor.matmul(
                                    pt[:, h * NB:(h + 1) * NB],
                                    ws8_sb[:kw, c, j * OT:j * OT + wj],
                                    xt[:kw, c:c + 1, co:co + NB],
                                    start=(c == 0), stop=False)
                        nc.tensor.matmul(
                            pt[:, h * NB:(h + 1) * NB],
                            wb8_sb[:R_lens[j], j * OT:j * OT + wj],
                            xt[:R_lens[j], n_sc + j:n_sc + j + 1, co:co + NB],
                            start=False, stop=True)
                    dst = (y1p[j // 2][:wj, j % 2, :] if j < 2 * n_pr
                           else y1l[:])
                    if j % 2 == 0:
                        nc.vector.tensor_scalar(dst, pt[:],
                                                b1_sb[:wj, j:j + 1], 0.0,
                                                op0=ALU.add, op1=ALU.max)
                    else:
                        nc.scalar.activation(dst, pt[:], AF.Relu,
                                             bias=b1_sb[:wj, j:j + 1])

                # ---- layer 2 (fp8 DoubleRow over k-chunk pairs) ----
                p2 = ps2.tile([H, 2 * NB], F32, tag="l2", name="p2")
                for q in range(n_pr):
                    for h in range(2):
                        nc.tensor.matmul(
                            p2[:, h * NB:(h + 1) * NB],
                            w2_sb[:, :, q * H:(q + 1) * H],
                            y1p[q][:, :, h * NB:(h + 1) * NB],
                            start=(q == 0), stop=(q == n_pr - 1 and not lw),
                            perf_mode=DR)
                if lw:
                    for h in range(2):
                        nc.tensor.matmul(
                            p2[:, h * NB:(h + 1) * NB], w26_sb[:],
                            y1l[:, h * NB:(h + 1) * NB],
                            start=False, stop=True)
                # undo the fp8 weight pre-scales (x16 from W1, x8 from W2)
                y2 = yp.tile([H, 2 * NB], F16, tag="y2")
                nc.scalar.activation(y2[:], p2[:], AF.Relu,
                                     bias=b2_sb[:, 0:1],
                                     scale=1.0 / (WSCALE * W2SCALE))
                y2_tiles.append(y2)

                # ---- batched layer 3 + log_softmax for 4 chunks, col-tiled
                # into one PSUM bank at partition offsets 0/32/64/96.
                # Emitted one pair LATE: the exp->lse->ln->sub chain is 3
                # serial engine hops, and emitting it right after its y2s
                # would stall the strict-FIFO ACT/DVE queues (and the L1
                # evictions queued behind it) for the chain's latency.
                # Deferring one pair lets independent eviction work fill
                # the queues while the chain's inputs are already ready.
                todo = []
                if p >= 2 and p % 2 == 0:
                    todo.append(p // 2 - 1)
                if p == n_ch // 2 - 1:
                    todo.append(n_ch // 4 - 1)
                for g in todo:
                    pg = ps3.tile([EP, NB], F32, tag="l3", name="pg")
                    # b3 broadcast into all slots via a K=1 ones-matmul
                    # (start=True clears the bank; col MMs accumulate).
                    nc.tensor.matmul(pg[:], b3r_sb[0:1, :], onec_sb[0:1, :],
                                     start=True, stop=False)
                    srcs = [(y2_tiles[2 * g], 0), (y2_tiles[2 * g], 1),
                            (y2_tiles[2 * g + 1], 0), (y2_tiles[2 * g + 1], 1)]
                    for tl, (y2t, h) in enumerate(srcs):
                        nc.tensor.matmul(
                            pg[32 * tl:32 * tl + C, :], w3_sb[:],
                            y2t[:, h * NB:(h + 1) * NB],
                            start=False, stop=(tl == 3),
                            tile_position=(0, 32 * tl))
                    ex = ep.tile([EP, NB], F16, tag="ex")
                    nc.scalar.activation(ex[:], pg[:], AF.Exp)
                    ps_l = ps4.tile([EP, NB], F32, tag="lse", name="ps_l")
                    nc.tensor.matmul(ps_l[:], obd_sb[:], ex[:], start=True,
                                     stop=True)
                    ls = ep.tile([EP, NB], F32, tag="ls")
                    nc.scalar.activation(ls[:], ps_l[:], AF.Ln)
                    o = ep.tile([EP, NB], F16, tag="o")
                    nc.vector.tensor_tensor(o[:], pg[:], ls[:],
                                            op=ALU.subtract)
                    for tl in range(GRP):
                        t = GRP * g + tl
                        ring = nc.sync if tl % 2 == 0 else nc.scalar
                        ring.dma_start(
                            out_d[:, t * NB:(t + 1) * NB],
                            o[32 * tl:32 * tl + C, :])

    nc.compile()
    return nc


_CACHE = {}


def _prepare(x, W1, b1, W2, b2, W3, b3, mask1, mask2, mask3):
    B, D1 = x.shape
    H = W2.shape[0]
    C = W3.shape[0]
    assert B % N_CORES == 0
    Bc = B // N_CORES

    S, R_list = _decompose_mask1(np.asarray(mask1))
    R_lens = [len(r) for r in R_list]
    n_blk = len(R_list)
    blk_w = [min(OT, D1 - j * OT) for j in range(n_blk)]
    P_pad = max(R_lens + [1])
    nS = len(S)
    n_sc = (nS + 127) // 128
    sc_w = -(-nS // n_sc)
    n_lane = n_sc + n_blk
    EP = 32 * (GRP - 1) + C

    Wm1 = (np.asarray(W1) * np.asarray(mask1)).astype(np.float32)
    Wm2 = (np.asarray(W2) * np.asarray(mask2)).astype(np.float32)
    Wm3 = (np.asarray(W3) * np.asarray(mask3)).astype(np.float32)

    c16 = lambda a: np.asarray(a, dtype=np.float16)
    c8 = lambda a: np.asarray(a, dtype=NP8)

    # ---- weights ----
    ws8 = np.zeros((sc_w, n_sc, D1), np.float32)
    for c in range(n_sc):
        rows = S[c * sc_w:(c + 1) * sc_w]
        ws8[:len(rows), c, :] = Wm1[:, rows].T * WSCALE
    wb8 = np.zeros((P_pad, D1), np.float32)
    for j, R in enumerate(R_list):
        wb8[:len(R), j * OT:j * OT + blk_w[j]] = \
            Wm1[j * OT:j * OT + blk_w[j], R].T * WSCALE
    n_pr = n_blk // 2
    lw = blk_w[-1] if n_blk % 2 else 0
    w2 = np.zeros((OT, 2, n_pr * H), np.float32)
    for q in range(n_pr):
        for o in range(2):
            j = 2 * q + o
            w2[:blk_w[j], o, q * H:(q + 1) * H] = \
                Wm2[:, j * OT:j * OT + blk_w[j]].T * W2SCALE
    w26 = np.zeros((max(lw, 1), H), np.float32)
    if lw:
        w26[:lw, :] = Wm2[:, (n_blk - 1) * OT:].T * W2SCALE
    w3 = np.ascontiguousarray(Wm3.T)                      # [H, C]
    b1p = np.zeros((OT, n_blk), np.float32)
    for j in range(n_blk):
        b1p[:blk_w[j], j] = WSCALE * \
            np.asarray(b1, np.float32)[j * OT:j * OT + blk_w[j]]
    b2p = np.asarray(b2, np.float32).reshape(H, 1)
    b3r = np.zeros((1, EP), np.float32)
    obd = np.zeros((EP, EP), np.float32)
    for tl in range(GRP):
        b3r[0, 32 * tl:32 * tl + C] = np.asarray(b3, np.float32)
        for m in range(32 * tl, min(32 * tl + 32, EP)):
            obd[32 * tl:32 * tl + C, m] = 1.0
    onec = np.ones((1, NB), np.float32)

    # ---- x slabs: one array per superchunk [NC, P_pad, n_lane, S_w] ----
    xT = np.asarray(x, np.float32).T                      # [D1, B] view
    xarrs = [np.zeros((N_CORES, P_pad, n_lane, S_w), NP8) for S_w in SCS]

    def fill_lane(lane, rows):
        data = c8(xT[rows]).reshape(len(rows), N_CORES, Bc)
        start = 0
        for s, S_w in enumerate(SCS):
            xarrs[s][:, :len(rows), lane, :] = \
                data[:, :, start:start + S_w].transpose(1, 0, 2)
            start += S_w

    for c in range(n_sc):
        fill_lane(c, S[c * sc_w:(c + 1) * sc_w])
    for j, R in enumerate(R_list):
        fill_lane(n_sc + j, R)

    meta = dict(nS=nS, R_lens=R_lens, P_pad=P_pad, Bc=Bc, D1=D1, H=H, C=C)
    key = (B, D1, H, C, nS, tuple(R_lens))
    if key not in _CACHE:
        _CACHE[key] = _build_program(meta)
    nc = _CACHE[key]

    in_maps = []
    for cidx in range(N_CORES):
        m = {f"x{s}": xarrs[s][cidx] for s in range(len(SCS))}
        m.update({
            "ws8": c8(ws8), "wb8": c8(wb8), "w2": c8(w2), "w3": c16(w3),
            "b1": b1p, "b2": b2p,
            "b3r": c16(b3r), "onec": c16(onec), "obd": c16(obd),
        })
        if lw:
            m["w26"] = c8(w26)
        in_maps.append(m)
    return nc, in_maps, meta


def _assemble(results, meta):
    outs = [np.ascontiguousarray(results[c]["out"].T).astype(np.float32)
            for c in range(N_CORES)]
    return np.concatenate(outs, axis=0)


def kernel(**inputs):
    nc, in_maps, meta = _prepare(**inputs)
    res = bass_utils.run_bass_kernel_spmd(nc, in_maps,
                                          core_ids=list(range(N_CORES)))
    return _assemble(res.results, meta)


def kernel_traced(tmpdir=None, **inputs):
    """Same as kernel() but with NTFF profiling; returns (output, results)."""
    nc, in_maps, meta = _prepare(**inputs)
    res = bass_utils.run_bass_kernel_spmd(nc, in_maps,
                                          core_ids=list(range(N_CORES)),
                                          trace=True, tmpdir=tmpdir)
    return _assemble(res.results, meta), res
